# revision 36
# baseline (speedup 1.0000x reference)
"""BitAttention TRN2 kernel: 8-core SPMD (DP over batch x TP over kv-heads).

Self-contained: hardcodes shapes B=2, S=2048, D=2048, H=16, KH=4.
Core r: batch b = r//4, kv-head kh = r%4, output stripe qq = r%4.

Math (forward-equivalent to the reference):
  - linear_bit = rms_norm -> per-row int8 act quant -> ternary weight quant
    -> matmul. Act-quant scale 127/max|xn| has rms self-cancel: the quantized
    ints are round(x*127/mx); rms enters only the per-token dequant scale.
  - Activations quantize straight to f16 with a +1536 offset (f16 ulp is 1 on
    [1024,2048) so the convert rounds half-to-even like jnp.round); the
    constant 1536 offset is removed inside the matmul by accumulating
    -1536*colsum(W) built from two exact f16 hi/lo matmuls.
  - Ternary weights quantize via round(clip(w*0.5/thr,-1,1)) (equivalent to
    round(tanh)), computed with the same +1536 f16 rounding trick.
  - The reference einsum sums the query-head group axis, so Q's 16 heads
    collapse to 4: group-sum the ternary w_q head blocks (ints in [-4,4]).
  - Scale folding: both 1/sqrt(HD) plus the wq/wk arctanh scales fold into
    the q rope tables (rope is linear); the wv scale cancels through the
    output rms-norm; the wo scale folds into the final dequant.
  - Attention runs transposed (S^T = K Q^T per key block) so softmaxed probs
    feed P^T directly into PV matmuls with no PE transposes; the softmax
    denominator comes from an extra all-ones column in the PV matmul.
    No max subtraction (logits are O(1) by construction).
  - Weight quantization work is sharded: batch-pair cores split w_q/w_k/w_v
    by row blocks, all 8 cores split w_o by columns; ternarized weights are
    exchanged with AllGather.
  - All tensor-engine transposes are done by DMA-transpose (f16/bf16).
  - The attention-out exchange is an AllToAll within each batch group (4
    cores), so each received slot is one kv-head's slice, concatenating
    directly into the KVD axis.
"""
import numpy as np
from contextlib import ExitStack

import concourse.bass as bass
import concourse.bacc as bacc
import concourse.mybir as mybir
import concourse.tile as tile
from concourse.bass_utils import run_bass_kernel_spmd

B, S, D = 2, 2048, 2048
H, KH = 16, 4
HD = D // H          # 128
KVD = KH * HD        # 512
NB = S // 128        # 16 token blocks
SQ = S // 4          # 512 tokens per output stripe
EPS = 1e-8
MAGIC = float(1.5 * 2 ** 23)
M16 = 1536.0
ATANH05 = 0.5493061443340549      # arctanh(0.5)
NEG = -3.4e38
F32 = mybir.dt.float32
BF16 = mybir.dt.bfloat16
F16 = mybir.dt.float16
AX = mybir.AxisListType
OP = mybir.AluOpType
AF = mybir.ActivationFunctionType

_cache = {}


def _pt_off(kb, causal):
    if causal:
        return 2048 * kb - 64 * kb * (kb - 1)
    return 2048 * kb


def build(causal: bool, local_cc: bool = False):
    nc = bacc.Bacc()
    x_d = nc.dram_tensor("x", [S, D], F32, kind="ExternalInput")
    xs_d = nc.dram_tensor("xstat", [SQ, D], F32, kind="ExternalInput")
    wq_d = nc.dram_tensor("wq", [D // 2, KVD], F32, kind="ExternalInput")   # row-shard
    wk_d = nc.dram_tensor("wk", [D // 2, HD], F32, kind="ExternalInput")
    wv_d = nc.dram_tensor("wv", [D // 2, HD], F32, kind="ExternalInput")
    wo_d = nc.dram_tensor("wo", [KVD, D // 8], F32, kind="ExternalInput")   # col-shard
    cos_d = nc.dram_tensor("cos", [128, NB * HD // 2], F32, kind="ExternalInput")
    sin_d = nc.dram_tensor("sin", [128, NB * HD // 2], F32, kind="ExternalInput")
    y_d = nc.dram_tensor("y", [SQ, D], F32, kind="ExternalOutput")
    st_in = nc.dram_tensor("st_in", [1, 4], F32)
    sc_in = nc.dram_tensor("sc_in", [4, 128, 2], F32)
    sc_out = nc.dram_tensor("sc_out", [4, 4, 128, 2], F32)
    st_out = nc.dram_tensor("st_out", [1, 4], F32, addr_space="Shared")
    wg_in = nc.dram_tensor("wg_in", [8, 128, 3 * HD], F16)
    wg_out = nc.dram_tensor("wg_out", [2, 8, 128, 3 * HD], F16)
    wob_in = nc.dram_tensor("wob_in", [4, 128, 256], BF16)
    wob_out = nc.dram_tensor("wob_out", [8, 4, 128, 256], BF16, addr_space="Shared")
    cc_in = nc.dram_tensor("cc_in", [4, 8, 128, HD], F32)
    qsel_d = nc.dram_tensor("qsel", [128, 2], F32, kind="ExternalInput")
    cc_out = nc.dram_tensor("cc_out", [4, 8, 128, HD], F32)

    PTW = _pt_off(NB, causal)

    with tile.TileContext(nc) as tc, ExitStack() as ctx:
        cpool = ctx.enter_context(tc.tile_pool(name="const", bufs=1))
        sm = ctx.enter_context(tc.tile_pool(name="sm", bufs=1))
        wres = ctx.enter_context(tc.tile_pool(name="wres", bufs=1))

        # ---------- constants ----------
        cmT = cpool.tile([128, 128], F32, tag="cmT")
        if causal:
            nc.gpsimd.memset(cmT[:], 0.0)
            nc.gpsimd.affine_select(out=cmT[:], in_=cmT[:], compare_op=OP.is_ge,
                                    fill=NEG, base=0, pattern=[[1, 128]],
                                    channel_multiplier=-1)
        ones_f16 = cpool.tile([128, 1], F16, tag="o16")
        nc.any.memset(ones_f16[:], 1.0)
        ones_bf = cpool.tile([128, 1], BF16, tag="obf")
        nc.any.memset(ones_bf[:], 1.0)
        ones_c = cpool.tile([128, 1], F32, tag="onc")
        nc.any.memset(ones_c[:], 1.0)
        ones_r = cpool.tile([1, 128], F32, tag="onr")
        nc.any.memset(ones_r[:], 1.0)
        n192 = cpool.tile([128, 128], F16, tag="n192")
        nc.any.memset(n192[:], -192.0)
        n12 = cpool.tile([128, 128], F16, tag="n12")
        nc.any.memset(n12[:], -12.0)
        inv_n = cpool.tile([128, 4], F32, tag="invn")
        for j, numel in enumerate([D * D, KVD * D, KVD * D, D * KVD]):
            nc.any.memset(inv_n[:, j:j + 1], 1.0 / numel)
        sqscr = cpool.tile([128, D], BF16, tag="sqscr")
        qsel = cpool.tile([128, 2], F32, tag="qsel")
        nc.sync.dma_start(qsel[:], qsel_d[:])
        cos_kb = cpool.tile([128, NB, HD // 2], BF16, tag="coskb")
        sin_kb = cpool.tile([128, NB, HD // 2], BF16, tag="sinkb")
        cos_qb = cpool.tile([128, NB, HD // 2], BF16, tag="cosqb")
        sin_qb = cpool.tile([128, NB, HD // 2], BF16, tag="sinqb")

        # persistent smalls
        pr = sm.tile([128, 4], F32, tag="pr")
        st_sb = sm.tile([1, 4], F32, tag="st_sb")
        st2_sb = sm.tile([1, 4], F32, tag="st2_sb")
        totals = sm.tile([128, 4], F32, tag="totals")
        s4 = sm.tile([128, 4], F32, tag="s4")
        hi4 = sm.tile([128, 4], F32, tag="hi4")
        a4 = sm.tile([128, 4], F32, tag="a4")
        aqk = sm.tile([128, 1], F32, tag="aqk")
        mxs = sm.tile([128, 4], F32, tag="mxs")
        ssqs = sm.tile([128, 4], F32, tag="ssqs")
        sx_sb = sm.tile([128, 4, 2], F32, tag="sx_sb")
        sd_all = sm.tile([128, 16, 2], F32, tag="sd_all")

        # persistent quantized weights
        wqkv = wres.tile([128, NB, 3 * HD], F16, tag="wqkv", name="wqkv")

        # x streaming pool (closed before attention) + attention-input tiles
        atl = ctx.enter_context(tc.tile_pool(name="atl", bufs=1))
        xqTp_cm = tc.tile_pool(name="xqTp", bufs=1)
        xqTp = xqTp_cm.__enter__()
        xph_cm = tc.tile_pool(name="xph", bufs=1)
        xph = xph_cm.__enter__()
        qT = atl.tile([128, NB, 128], BF16, tag="qT", name="qT")
        kT = atl.tile([128, NB, 128], BF16, tag="kT", name="kT")
        qkv_all = sm.tile([128, NB, 3 * HD], BF16, tag="qkv_all", name="qkv_all")
        vaug = sm.tile([128, NB, 132], BF16, tag="vaug", name="vaug")
        nc.any.memset(vaug[:], 1.0)
        qr = xqTp.tile([128, NB, HD], BF16, tag="qr", name="qr")
        kr = xqTp.tile([128, NB, HD], BF16, tag="kr", name="kr")
        csr = xqTp.tile([1, 3 * HD], F32, tag="csr", name="csr")
        csbc = xqTp.tile([128, 3 * HD], F32, tag="csbc", name="csbc")
        tcs = xqTp.tile([128, 3 * HD], F32, tag="tcs", name="tcs")
        hq = xqTp.tile([128, 3 * HD], F16, tag="hq", name="hq")
        lq = xqTp.tile([128, 3 * HD], F16, tag="lq", name="lq")

        qchs = [None] * 4
        xbss = [None] * 4
        xqTc = [None] * 4

        def prep_dma(ci, eng):
            xbs = []
            for ib in range(4):
                i = 4 * ci + ib
                xb = xph.tile([128, D], F32, tag="xb", bufs=3, name=f"xb{ci}_{ib}")
                eng.dma_start(xb[:], x_d[i * 128:(i + 1) * 128, :])
                xbs.append(xb)
            xbss[ci] = xbs

        def prep_quant(ci):
            qch = xph.tile([128, 4, D], F16, tag="qch", bufs=2, name=f"qch{ci}")
            qchs[ci] = qch
            for ib in range(4):
                half_quant(qch, ib, xbss[ci][ib], 4 * ci + ib)

        def prep_tp(ci, eng):
            xqTc[ci] = xqTp.tile([128, 64, 128], F16, tag="xqTc", bufs=2,
                                 name=f"xqTc{ci}")
            eng.dma_start_transpose(
                xqTc[ci][:], qchs[ci][:].rearrange("p a b -> p (a b)"))

        def xstat_load():
            for i in range(4):
                xsb = xph.tile([128, D], F32, tag="xb", bufs=3, name=f"xs{i}")
                nc.sync.dma_start(xsb[:], xs_d[i * 128:(i + 1) * 128, :])
                nc.vector.tensor_reduce(mxs[:, i:i + 1], xsb[:], axis=AX.X,
                                        op=OP.max, apply_absolute_value=True)
                nc.scalar.activation(sqscr[:], xsb[:], AF.Square,
                                     accum_out=ssqs[:, i:i + 1])

        def xstat_fin():
            mean4 = sm.tile([128, 4], F32, tag="mean4")
            nc.vector.tensor_scalar(mean4[:], ssqs[:], 1.0 / D, EPS, op0=OP.mult, op1=OP.add)
            lg4 = sm.tile([128, 4], F32, tag="lg4")
            nc.scalar.activation(lg4[:], mean4[:], AF.Ln)
            r4 = sm.tile([128, 4], F32, tag="r4")
            nc.scalar.activation(r4[:], lg4[:], AF.Exp, scale=-0.5)
            nt4 = sm.tile([128, 4], F32, tag="nt4")
            m127 = sm.tile([128, 4], F32, tag="m127")
            nc.vector.tensor_scalar(m127[:], mxs[:], 1.0 / 127.0, None, op0=OP.mult)
            smul4 = sm.tile([128, 4], F32, tag="smul4")
            nc.vector.reciprocal(smul4[:], m127[:])
            nc.vector.tensor_tensor(nt4[:], m127[:], smul4[:], op=OP.mult)
            nc.vector.tensor_scalar(nt4[:], nt4[:], -1.0, 2.0, op0=OP.mult, op1=OP.add)
            nc.vector.tensor_tensor(smul4[:], smul4[:], nt4[:], op=OP.mult)
            deq4 = sm.tile([128, 4], F32, tag="deq4")
            nc.vector.tensor_tensor(deq4[:], mxs[:], r4[:], op=OP.mult)
            nc.vector.tensor_scalar(deq4[:], deq4[:], 1.0 / 127.0, None, op0=OP.mult)
            nc.vector.tensor_copy(sx_sb[:, :, 0], smul4[:])
            nc.vector.tensor_copy(sx_sb[:, :, 1], deq4[:])
            nc.sync.dma_start(sc_in.ap().rearrange("i p c -> p i c"), sx_sb[:])
            if local_cc:
                nc.sync.dma_start(sc_out.ap()[0], sc_in.ap())
            else:
                nc.gpsimd.collective_compute(
                    "AllGather", OP.bypass,
                    replica_groups=[[0, 1, 2, 3], [4, 5, 6, 7]],
                    ins=[sc_in.ap().opt()], outs=[sc_out.ap().opt()])
            nc.sync.dma_start(sd_all[:], sc_out.ap().rearrange("s i p c -> p (s i) c"))

        def half_quant(qch, ib, xb, i):
            hw = D // 2
            nc.vector.tensor_scalar(qch[:, ib, 0:hw], xb[:, 0:hw],
                                    sd_all[:, i, 0:1], M16, op0=OP.mult, op1=OP.add)
            nc.scalar.activation(qch[:, ib, hw:D], xb[:, hw:D], AF.Copy,
                                 bias=M16, scale=sd_all[:, i, 0:1])

        def rope_chunk(ci):
            hh = HD // 2
            cs = slice(4 * ci, 4 * ci + 4)
            for src0, cosb, sinb, dst in ((0, cos_qb, sin_qb, qr),
                                          (HD, cos_kb, sin_kb, kr)):
                ev = qkv_all[:, cs, src0:src0 + hh]
                od = qkv_all[:, cs, src0 + hh:src0 + HD]
                t1 = xph.tile([128, 4, hh], BF16, tag="t1", bufs=2)
                t2 = xph.tile([128, 4, hh], BF16, tag="t2", bufs=2)
                nc.vector.tensor_tensor(t1[:], ev, cosb[:, cs, :], op=OP.mult)
                nc.vector.tensor_tensor(t2[:], od, sinb[:, cs, :], op=OP.mult)
                nc.vector.tensor_tensor(dst[:, cs, 0:hh], t1[:], t2[:], op=OP.subtract)
                t3 = xph.tile([128, 4, hh], BF16, tag="t1", bufs=2)
                t4 = xph.tile([128, 4, hh], BF16, tag="t2", bufs=2)
                nc.vector.tensor_tensor(t3[:], ev, sinb[:, cs, :], op=OP.mult)
                nc.vector.tensor_tensor(t4[:], od, cosb[:, cs, :], op=OP.mult)
                nc.vector.tensor_tensor(dst[:, cs, hh:HD], t3[:], t4[:], op=OP.add)
            nc.sync.dma_start_transpose(
                qT[:, cs, :], qr[:, cs, :].rearrange("p a b -> p (a b)"))
            nc.sync.dma_start_transpose(
                kT[:, cs, :], kr[:, cs, :].rearrange("p a b -> p (a b)"))

        with tc.tile_pool(name="cstage", bufs=1) as cstage:
          with tc.tile_pool(name="wph", bufs=1) as wph:
            # ---- all input DMAs up front (SP in readiness order) ----
            cosf = cstage.tile([128, NB, HD // 2], F32, tag="cosf")
            sinf = cstage.tile([128, NB, HD // 2], F32, tag="sinf")
            wq_sb = wph.tile([128, 8, KVD], F32, tag="wq_sb")
            wk_sb = wph.tile([128, 8, HD], F32, tag="wk_sb")
            wv_sb = wph.tile([128, 8, HD], F32, tag="wv_sb")
            wo_sb = wph.tile([128, 4, 256], F32, tag="wo_sb")
            nc.sync.dma_start(wq_sb[:], wq_d.ap().rearrange("(j p) c -> p j c", p=128))
            nc.sync.dma_start(wk_sb[:], wk_d.ap().rearrange("(j p) c -> p j c", p=128))
            nc.sync.dma_start(wv_sb[:], wv_d.ap().rearrange("(j p) c -> p j c", p=128))
            nc.sync.dma_start(wo_sb[:], wo_d.ap().rearrange("(c p) d -> p c d", p=128))
            # ---- pass1 |w| row sums (before x stats: heads the longer path) ----
            nc.vector.tensor_reduce(pr[:, 0:1], wq_sb[:].rearrange("p a b -> p (a b)"),
                                    axis=AX.X, op=OP.add, apply_absolute_value=True)
            nc.vector.tensor_reduce(pr[:, 1:2], wk_sb[:].rearrange("p a b -> p (a b)"),
                                    axis=AX.X, op=OP.add, apply_absolute_value=True)
            nc.vector.tensor_reduce(pr[:, 2:3], wv_sb[:].rearrange("p a b -> p (a b)"),
                                    axis=AX.X, op=OP.add, apply_absolute_value=True)
            nc.vector.tensor_reduce(pr[:, 3:4], wo_sb[:].rearrange("p a b -> p (a b)"),
                                    axis=AX.X, op=OP.add, apply_absolute_value=True)

            xstat_load()

            # ---- weight stats reduce + exchange (SP) ----
            with tc.tile_pool(name="psst", bufs=2, space="PSUM") as psst:
                pcol = psst.tile([1, 4], F32, tag="st")
                nc.tensor.matmul(pcol[:], ones_c[:], pr[:], start=True, stop=True)
                nc.vector.tensor_copy(st_sb[:], pcol[:])
                nc.sync.dma_start(st_in[:], st_sb[:])
                if local_cc:
                    nc.sync.dma_start(st_out.ap(), st_in.ap())
                else:
                    nc.gpsimd.collective_compute(
                        "AllReduce", OP.add, replica_groups=[list(range(8))],
                        ins=[st_in.ap().opt()], outs=[st_out.ap().opt()])
                nc.sync.dma_start(st2_sb[:], st_out[:])
                bc = psst.tile([128, 4], F32, tag="st")
                nc.tensor.matmul(bc[:], ones_r[:], st2_sb[:], start=True, stop=True)
                nc.vector.tensor_copy(totals[:], bc[:])

            xbs = []
            for ib in range(2):
                xb = xph.tile([128, D], F32, tag="xb", bufs=3, name=f"xb0_{ib}")
                nc.sync.dma_start(xb[:], x_d[ib * 128:(ib + 1) * 128, :])
                xbs.append(xb)
            xstat_fin()
            for ib in range(2, 4):
                xb = xph.tile([128, D], F32, tag="xb", bufs=3, name=f"xb0_{ib}")
                nc.sync.dma_start(xb[:], x_d[ib * 128:(ib + 1) * 128, :])
                xbs.append(xb)
            xbss[0] = xbs
            prep_quant(0)
            prep_dma(1, nc.sync)
            prep_quant(1)

            # ---- ternary thresholds and scales ----
            nc.vector.tensor_tensor(s4[:], totals[:], inv_n[:], op=OP.mult)
            thr2 = sm.tile([128, 4], F32, tag="thr2")
            nc.vector.tensor_scalar(thr2[:], s4[:], EPS, 2.0 * ATANH05,
                                    op0=OP.add, op1=OP.mult)
            nc.vector.reciprocal(hi4[:], thr2[:])
            ntp = sm.tile([128, 4], F32, tag="ntp")
            nc.vector.tensor_tensor(ntp[:], thr2[:], hi4[:], op=OP.mult)
            nc.vector.tensor_scalar(ntp[:], ntp[:], -1.0, 2.0, op0=OP.mult, op1=OP.add)
            nc.vector.tensor_tensor(hi4[:], hi4[:], ntp[:], op=OP.mult)
            num = sm.tile([128, 4], F32, tag="num")
            den = sm.tile([128, 4], F32, tag="den")
            rat = sm.tile([128, 4], F32, tag="rat")
            nc.vector.tensor_scalar(num[:], s4[:], 1.0, None, op0=OP.add)
            nc.vector.tensor_scalar(den[:], s4[:], -1.0, 1.0, op0=OP.mult, op1=OP.add)
            nc.vector.reciprocal(rat[:], den[:])
            nc.vector.tensor_tensor(ntp[:], den[:], rat[:], op=OP.mult)
            nc.vector.tensor_scalar(ntp[:], ntp[:], -1.0, 2.0, op0=OP.mult, op1=OP.add)
            nc.vector.tensor_tensor(rat[:], rat[:], ntp[:], op=OP.mult)
            nc.vector.tensor_tensor(rat[:], rat[:], num[:], op=OP.mult)
            lnr = sm.tile([128, 4], F32, tag="lnr")
            nc.scalar.activation(lnr[:], rat[:], AF.Ln)
            nc.vector.tensor_scalar(a4[:], lnr[:], 0.5, None, op0=OP.mult)
            nc.vector.tensor_tensor(aqk[:], a4[:, 0:1], a4[:, 1:2], op=OP.mult)
            nc.vector.tensor_scalar(aqk[:], aqk[:], 1.0 / HD, None, op0=OP.mult)

            # ---- ternary quantize shards (wq on DVE, rest on pool) ----
            wsh = wph.tile([128, 8, 3 * HD], F16, tag="wsh")
            wosh = wph.tile([128, 4, 256], BF16, tag="wosh")
            tq = wph.tile([128, 4, KVD], F16, tag="tq")
            tb16 = wph.tile([128, 1024], F16, tag="tb16")
            nc.vector.tensor_scalar(wq_sb[:], wq_sb[:], hi4[:, 0:1], 1.0,
                                    op0=OP.mult, op1=OP.min)
            for h in range(2):
                js = slice(4 * h, 4 * h + 4)
                nc.vector.tensor_scalar(tq[:], wq_sb[:, js, :], -1.0, M16,
                                        op0=OP.max, op1=OP.add)
                nc.vector.tensor_scalar(tq[:], tq[:], M16, None, op0=OP.subtract)
                nc.vector.tensor_tensor(wsh[:, js, 0:HD], tq[:, :, 0:HD],
                                        tq[:, :, HD:2 * HD], op=OP.add)
                nc.vector.scalar_tensor_tensor(wsh[:, js, 0:HD], tq[:, :, 2 * HD:3 * HD],
                                               1.0, wsh[:, js, 0:HD],
                                               op0=OP.mult, op1=OP.add)
                nc.vector.scalar_tensor_tensor(wsh[:, js, 0:HD], tq[:, :, 3 * HD:4 * HD],
                                               1.0, wsh[:, js, 0:HD],
                                               op0=OP.mult, op1=OP.add)
            tkv = tb16[:].rearrange("p (a b) -> p a b", a=8)
            nc.gpsimd.tensor_scalar(wk_sb[:], wk_sb[:], hi4[:, 1:2], 1.0,
                                    op0=OP.mult, op1=OP.min)
            nc.gpsimd.tensor_scalar(tkv, wk_sb[:], -1.0, M16, op0=OP.max, op1=OP.add)
            nc.gpsimd.tensor_scalar(wsh[:, :, HD:2 * HD], tkv, M16, None, op0=OP.subtract)
            nc.gpsimd.tensor_scalar(wv_sb[:], wv_sb[:], hi4[:, 2:3], 1.0,
                                    op0=OP.mult, op1=OP.min)
            nc.gpsimd.tensor_scalar(tkv, wv_sb[:], -1.0, M16, op0=OP.max, op1=OP.add)
            nc.gpsimd.tensor_scalar(wsh[:, :, 2 * HD:3 * HD], tkv, M16, None, op0=OP.subtract)
            two = tb16[:].rearrange("p (a b) -> p a b", a=4)
            nc.gpsimd.tensor_scalar(wo_sb[:], wo_sb[:], hi4[:, 3:4], 1.0,
                                    op0=OP.mult, op1=OP.min)
            nc.gpsimd.tensor_scalar(two, wo_sb[:], -1.0, M16, op0=OP.max, op1=OP.add)
            nc.gpsimd.tensor_scalar(wosh[:], two, M16, None, op0=OP.subtract)

            # ---- exchange ternary shards ----
            nc.sync.dma_start(wg_in.ap().rearrange("j p c -> p j c"), wsh[:])
            nc.sync.dma_start(wob_in.ap().rearrange("c p d -> p c d"), wosh[:])
            if local_cc:
                nc.sync.dma_start(wg_out.ap()[0], wg_in.ap())
            else:
                nc.gpsimd.collective_compute(
                    "AllGather", OP.bypass,
                    replica_groups=[[0, 4], [1, 5], [2, 6], [3, 7]],
                    ins=[wg_in.ap().opt()], outs=[wg_out.ap().opt()])
            for sgi in range(2):
                nc.sync.dma_start(
                    wqkv[:].rearrange("p (j s) c -> p j s c", s=2)[:, :, sgi, :],
                    wg_out.ap()[sgi].rearrange("j p c -> p j c"))
            prep_tp(0, nc.sync)
            prep_tp(1, nc.sync)
            nc.sync.dma_start(cosf[:], cos_d.ap().rearrange("p (i f) -> p i f", i=NB))
            nc.sync.dma_start(sinf[:], sin_d.ap().rearrange("p (i f) -> p i f", i=NB))
            nc.vector.tensor_copy(cos_kb[:], cosf[:])
            nc.vector.tensor_copy(sin_kb[:], sinf[:])
            nc.vector.tensor_scalar(cos_qb[:], cosf[:], aqk[:], None, op0=OP.mult)
            nc.vector.tensor_scalar(sin_qb[:], sinf[:], aqk[:], None, op0=OP.mult)
          # (wph closed: f32 weight shards freed)

        # ---------- x quantize/transpose fused with QKV ----------
        with tc.tile_pool(name="psc", bufs=3, space="PSUM") as psc:
            prep_dma(2, nc.sync)
            prep_quant(2)
            prep_tp(2, nc.sync)

            # csum of wqkv for the -1536 offset correction (f16 hi/lo split)
            csp = psc.tile([1, 3 * HD], F32, tag="cs", bufs=2)
            for j in range(NB):
                nc.tensor.matmul(csp[:], ones_f16[:], wqkv[:, j, :],
                                 start=(j == 0), stop=(j == NB - 1))
            nc.scalar.activation(csr[:], csp[:], AF.Copy)
            bcp = psc.tile([128, 3 * HD], F32, tag="cs", bufs=2)
            nc.tensor.matmul(bcp[:], ones_r[:], csr[:], start=True, stop=True)
            nc.scalar.activation(csbc[:], bcp[:], AF.Copy)
            nc.gpsimd.tensor_scalar(tcs[:], csbc[:], 1.0 / 16.0, MAGIC,
                                    op0=OP.mult, op1=OP.add)
            nc.gpsimd.tensor_scalar(tcs[:], tcs[:], MAGIC, None, op0=OP.subtract)
            nc.vector.tensor_copy(hq[:], tcs[:])
            nc.vector.scalar_tensor_tensor(lq[:], tcs[:], -16.0, csbc[:],
                                           op0=OP.mult, op1=OP.add)

            def qkv_chunk(ci):
                for ib in range(4):
                    i = 4 * ci + ib
                    pq = psc.tile([128, 3 * HD], F32, tag="mm", bufs=3)
                    for j in range(NB):
                        nc.tensor.matmul(pq[:], xqTc[ci][:, 16 * ib + j, :],
                                         wqkv[:, j, :], start=(j == 0), stop=False)
                    nc.tensor.matmul(pq[:], n192[:], hq[:], start=False, stop=False,
                                     skip_group_check=True)
                    nc.tensor.matmul(pq[:], n12[:], lq[:], start=False, stop=True,
                                     skip_group_check=True)
                    nc.vector.tensor_scalar(qkv_all[:, i, 0:2 * HD], pq[:, 0:2 * HD],
                                            sd_all[:, i, 1:2], None, op0=OP.mult)
                    nc.vector.tensor_scalar(vaug[:, i, 0:HD], pq[:, 2 * HD:3 * HD],
                                            sd_all[:, i, 1:2], None, op0=OP.mult)

            prep_dma(3, nc.sync)
            prep_quant(3)
            prep_tp(3, nc.sync)
            qkv_chunk(0)
            rope_chunk(0)
            qkv_chunk(1)
            rope_chunk(1)
            qkv_chunk(2)
            rope_chunk(2)
            qkv_chunk(3)
            rope_chunk(3)

        xph_cm.__exit__(None, None, None)
        xqTp_cm.__exit__(None, None, None)

        # ---------- attention + output projection ----------
        qTf = qT[:].rearrange("p a b -> p (a b)")
        with tc.tile_pool(name="attn", bufs=1) as attn, \
             tc.tile_pool(name="pss", bufs=3, space="PSUM") as pss, \
             tc.tile_pool(name="psv", bufs=3, space="PSUM") as psv, \
             tc.tile_pool(name="psy", bufs=2, space="PSUM") as psy:
            PT = attn.tile([128, PTW], BF16, tag="PT")
            wo_q = attn.tile([128, 4, D], BF16, tag="wo_q", name="wo_q")
            if local_cc:
                nc.scalar.dma_start(wob_out.ap()[0], wob_in.ap())
            else:
                nc.gpsimd.collective_compute(
                    "AllGather", OP.bypass, replica_groups=[list(range(8))],
                    ins=[wob_in.ap().opt()], outs=[wob_out.ap().opt()])
            for sgi in range(8):
                nc.scalar.dma_start(
                    wo_q[:].rearrange("p c (s d) -> p c s d", s=8)[:, :, sgi, :],
                    wob_out.ap()[sgi].rearrange("c p d -> p c d"))

            def scores(kb):
                qlo = 128 * kb if causal else 0
                c0 = qlo
                first = True
                while c0 < S:
                    cw = min(512, S - c0)
                    sp = pss.tile([128, 512], F32, tag="sc")
                    nc.tensor.matmul(sp[:, 0:cw], kT[:, kb, :], qTf[:, c0:c0 + cw],
                                     start=True, stop=True)
                    if causal and first:
                        nc.vector.tensor_tensor(sp[:, 0:128], sp[:, 0:128], cmT[:],
                                                op=OP.add)
                    nc.scalar.activation(PT[:, _pt_off(kb, causal) + c0 - qlo:
                                            _pt_off(kb, causal) + c0 - qlo + cw],
                                         sp[:, 0:cw], AF.Exp)
                    first = False
                    c0 += cw

            obufs = [None] * 4

            def pv(qb):
                if qb % 4 == 0:
                    obufs[qb // 4] = attn.tile([128, 4, HD], F32, tag="obuf",
                                               bufs=2, name=f"ob{qb // 4}")
                po = psv.tile([128, 132], F32, tag="po")
                nkb = qb + 1 if causal else NB
                for k2 in range(nkb):
                    qoff = (qb - k2) * 128 if causal else qb * 128
                    lhs = PT[:, _pt_off(k2, causal) + qoff:
                             _pt_off(k2, causal) + qoff + 128]
                    nc.tensor.matmul(po[:, 0:HD + 1], lhs, vaug[:, k2, 0:HD + 1],
                                     start=(k2 == 0), stop=(k2 == nkb - 1))
                rz = attn.tile([128, 1], F32, tag="rz", bufs=3)
                nc.vector.reciprocal(rz[:], po[:, HD:HD + 1])
                nc.vector.tensor_scalar(obufs[qb // 4][:, qb % 4, :], po[:, 0:HD],
                                        rz[:], None, op0=OP.mult)

            xos = [None] * 4

            def cc_ex(tb):
                nc.sync.dma_start(cc_in.ap()[tb, 0:4].rearrange("s p d -> p s d"),
                                  obufs[tb][:])
                nc.sync.dma_start(cc_in.ap()[tb, 4:8].rearrange("s p d -> p s d"),
                                  obufs[tb][:])
                if local_cc:
                    nc.sync.dma_start(cc_out.ap()[tb], cc_in.ap()[tb])
                else:
                    nc.gpsimd.collective_compute(
                        "AllToAll", OP.bypass, replica_groups=[list(range(8))],
                        ins=[cc_in.ap()[tb].opt()], outs=[cc_out.ap()[tb].opt()])
                xo8 = attn.tile([128, 8, HD], F32, tag="xo8", bufs=2, name=f"xo8_{tb}")
                nc.sync.dma_start(xo8[:], cc_out.ap()[tb].rearrange("s p d -> p s d"))
                xos[tb] = xo8

            def oproj(tb):
                xo8 = xos[tb]
                xsel = attn.tile([128, KVD], F32, tag="xsel", bufs=2)
                nc.gpsimd.tensor_scalar(xsel[:], xo8[:, 0:4, :].rearrange("p a b -> p (a b)"),
                                        qsel[:, 0:1], None, op0=OP.mult)
                xo = attn.tile([128, KVD], F32, tag="xo", bufs=2, name=f"xo{tb}")
                nc.vector.scalar_tensor_tensor(xo[:], xo8[:, 4:8, :].rearrange("p a b -> p (a b)"),
                                               qsel[:, 1:2], xsel[:],
                                               op0=OP.mult, op1=OP.add)
                mx2 = attn.tile([128, 1], F32, tag="mx2", bufs=2)
                nc.vector.tensor_reduce(mx2[:], xo[:], axis=AX.X, op=OP.max,
                                        apply_absolute_value=True)
                ssq2 = attn.tile([128, 1], F32, tag="ssq2", bufs=2)
                nc.scalar.activation(sqscr[:, 0:KVD], xo[:], AF.Square, accum_out=ssq2[:])
                mean2 = attn.tile([128, 1], F32, tag="mean2", bufs=2)
                nc.vector.tensor_scalar(mean2[:], ssq2[:], 1.0 / KVD, EPS,
                                        op0=OP.mult, op1=OP.add)
                lg2 = attn.tile([128, 1], F32, tag="lg2", bufs=2)
                nc.scalar.activation(lg2[:], mean2[:], AF.Ln)
                r2 = attn.tile([128, 1], F32, tag="r2", bufs=2)
                nc.scalar.activation(r2[:], lg2[:], AF.Exp, scale=-0.5)
                nt2 = attn.tile([128, 1], F32, tag="nt2", bufs=2)
                m2 = attn.tile([128, 1], F32, tag="m2", bufs=2)
                nc.vector.tensor_scalar(m2[:], mx2[:], 1.0 / 127.0, None, op0=OP.mult)
                sl2 = attn.tile([128, 1], F32, tag="sl2", bufs=2)
                nc.vector.reciprocal(sl2[:], m2[:])
                nc.vector.tensor_tensor(nt2[:], m2[:], sl2[:], op=OP.mult)
                nc.vector.tensor_scalar(nt2[:], nt2[:], -1.0, 2.0, op0=OP.mult, op1=OP.add)
                nc.vector.tensor_tensor(sl2[:], sl2[:], nt2[:], op=OP.mult)
                dqy = attn.tile([128, 1], F32, tag="dqy", bufs=2)
                nc.vector.tensor_tensor(dqy[:], mx2[:], r2[:], op=OP.mult)
                nc.vector.tensor_scalar(dqy[:], dqy[:], 1.0 / 127.0, None, op0=OP.mult)
                nc.vector.tensor_tensor(dqy[:], dqy[:], a4[:, 3:4], op=OP.mult)
                nc.vector.tensor_scalar(xo[:], xo[:], sl2[:], MAGIC,
                                        op0=OP.mult, op1=OP.add)
                qo = attn.tile([128, KVD], BF16, tag="qo", bufs=2)
                nc.scalar.activation(qo[:], xo[:], AF.Copy, bias=-MAGIC)
                xoT = attn.tile([128, 4, 128], BF16, tag="xoT", bufs=2)
                nc.scalar.dma_start_transpose(xoT[:], qo[:])
                y_sb = attn.tile([128, D], F32, tag="ysb", bufs=2)
                for oc in range(4):
                    py = psy.tile([128, 512], F32, tag="my")
                    for jc in range(4):
                        nc.tensor.matmul(py[:], xoT[:, jc, :],
                                         wo_q[:, jc, oc * 512:(oc + 1) * 512],
                                         start=(jc == 0), stop=(jc == 3))
                    if oc % 2 == 0:
                        nc.scalar.activation(y_sb[:, oc * 512:(oc + 1) * 512], py[:],
                                             AF.Copy, scale=dqy[:])
                    else:
                        nc.vector.tensor_scalar(y_sb[:, oc * 512:(oc + 1) * 512],
                                                py[:], dqy[:], None, op0=OP.mult)
                nc.sync.dma_start(y_d[tb * 128:(tb + 1) * 128, :], y_sb[:])

            def post_pv(qb):
                # at quarter completion: launch the exchange; run the PREVIOUS
                # quarter's output projection (its data has long arrived)
                if qb % 4 == 3:
                    tb = qb // 4
                    cc_ex(tb)
                    if tb >= 1:
                        oproj(tb - 1)

            if causal:
                scores(0)
                for kb in range(1, NB):
                    scores(kb)
                    pv(kb - 1)
                    post_pv(kb - 1)
                pv(NB - 1)
                post_pv(NB - 1)
            else:
                for kb in range(NB):
                    scores(kb)
                for qb in range(NB):
                    pv(qb)
                    post_pv(qb)
            oproj(3)
    nc.compile()
    return nc


def _rope_perm():
    p = np.empty(HD, np.int64)
    p[:HD // 2] = np.arange(0, HD, 2)
    p[HD // 2:] = np.arange(1, HD, 2)
    return p


def qsel_host(b):
    q = np.zeros((128, 2), np.float32)
    q[:, b] = 1.0
    return q


def _prep_inputs(inputs):
    x = np.ascontiguousarray(np.asarray(inputs["x"], np.float32))
    w_q = np.asarray(inputs["w_q"], np.float32)
    w_k = np.asarray(inputs["w_k"], np.float32)
    w_v = np.asarray(inputs["w_v"], np.float32)
    w_o = np.asarray(inputs["w_o"], np.float32)
    cos = np.asarray(inputs["freq_cos"], np.float32)
    sin = np.asarray(inputs["freq_sin"], np.float32)
    # token-block layout: row p, cols (i, f) = table[i*128 + p, f]
    cos_ar = np.ascontiguousarray(
        cos.reshape(NB, 128, HD // 2).transpose(1, 0, 2).reshape(128, -1))
    sin_ar = np.ascontiguousarray(
        sin.reshape(NB, 128, HD // 2).transpose(1, 0, 2).reshape(128, -1))
    perm = _rope_perm()
    woT = np.ascontiguousarray(w_o.T)                      # [KVD, D]
    in_maps = []
    jrows = np.arange(D) // 128 % 2
    for r in range(8):
        b, kh = r // 4, r % 4
        heads = [g * KH + kh for g in range(4)]
        wq_sel = w_q.reshape(H, HD, D)[heads][:, perm, :]  # [4,128,D]
        wqT = np.ascontiguousarray(wq_sel.reshape(4 * HD, D).T)   # [D, 512]
        wkT = np.ascontiguousarray(w_k[kh * HD:(kh + 1) * HD][perm].T)  # [D,128]
        wvT = np.ascontiguousarray(w_v[kh * HD:(kh + 1) * HD].T)        # [D,128]
        sel = jrows == b
        in_maps.append({
            "x": x[b],
            "xstat": np.ascontiguousarray(x[b][kh * SQ:(kh + 1) * SQ]),
            "wq": np.ascontiguousarray(wqT[sel]),
            "wk": np.ascontiguousarray(wkT[sel]),
            "wv": np.ascontiguousarray(wvT[sel]),
            "wo": np.ascontiguousarray(woT[:, r * 256:(r + 1) * 256]),
            "cos": cos_ar, "sin": sin_ar,
            "qsel": qsel_host(b),
        })
    return in_maps


def _gains_trivial(inputs):
    return all(np.all(np.asarray(inputs[g]) == 1.0)
               for g in ("g_q", "g_k", "g_v", "g_o"))


def _numpy_fallback(inputs):
    """Faithful numpy reimplementation (slow); used only for unexpected configs."""
    x = np.asarray(inputs["x"], np.float32)
    cos, sin = (np.asarray(inputs[k], np.float32) for k in ("freq_cos", "freq_sin"))
    causal = int(np.asarray(inputs["causal"]))

    def rms(t, g):
        n = t * (1.0 / np.sqrt(np.mean(t * t, -1, keepdims=True, dtype=np.float32) + EPS))
        return (g * n).astype(np.float32)

    def actq(t):
        scale = 127.0 / np.clip(np.max(np.abs(t), -1, keepdims=True), 1e-4, None)
        q = np.round(t * scale)
        return np.clip(q, -128, 127) / scale

    def ternq(w):
        s = np.mean(np.abs(w), dtype=np.float32)
        return np.round(np.tanh(w / (s + EPS))) * np.arctanh(s)

    def lin(t, w, g):
        return actq(rms(t, g)).astype(np.float32) @ ternq(np.asarray(w, np.float32)).T

    Bb, Ss, Dd = x.shape
    q = lin(x, inputs["w_q"], np.asarray(inputs["g_q"], np.float32)).reshape(Bb, Ss, H, HD)
    k = lin(x, inputs["w_k"], np.asarray(inputs["g_k"], np.float32)).reshape(Bb, Ss, KH, HD)
    v = lin(x, inputs["w_v"], np.asarray(inputs["g_v"], np.float32)).reshape(Bb, Ss, KH, HD)

    def rope(t):
        t2 = t.reshape(*t.shape[:-1], -1, 2)
        c = cos[None, :, None, :]
        s_ = sin[None, :, None, :]
        o0 = t2[..., 0] * c - t2[..., 1] * s_
        o1 = t2[..., 0] * s_ + t2[..., 1] * c
        return np.stack([o0, o1], -1).reshape(t.shape).astype(np.float32)

    q, k = rope(q), rope(k)
    scale = np.float32(HD ** 0.5)
    q = q.transpose(0, 2, 1, 3) / scale
    k = k.transpose(0, 2, 1, 3)
    v = v.transpose(0, 2, 1, 3)
    qg = q.reshape(Bb, 4, KH, Ss, HD).sum(1)
    sc = np.einsum("bhnd,bhsd->bhns", qg, k).astype(np.float32)
    if causal:
        mask = np.tril(np.ones((Ss, Ss), bool))
        sc = np.where(mask[None, None], sc, np.float32(np.finfo(np.float32).min))
    sc = sc / scale
    sc = sc - sc.max(-1, keepdims=True)
    p = np.exp(sc)
    p /= p.sum(-1, keepdims=True)
    out = np.einsum("bhns,bhsd->bnhd", p, v).reshape(Bb, Ss, KVD)
    return lin(out, inputs["w_o"], np.asarray(inputs["g_o"], np.float32))


def kernel(**inputs):
    x = np.asarray(inputs["x"])
    if x.shape != (B, S, D) or not _gains_trivial(inputs):
        return _numpy_fallback(inputs)
    causal = bool(int(np.asarray(inputs["causal"])))
    key = ("bitattn", causal)
    if key not in _cache:
        _cache[key] = build(causal)
    nc = _cache[key]
    in_maps = _prep_inputs(inputs)
    res = run_bass_kernel_spmd(nc, in_maps, core_ids=list(range(8)))
    y = np.empty((B, S, D), np.float32)
    for r in range(8):
        b, qq = r // 4, r % 4
        for tb in range(4):
            blk = 4 * tb + qq
            y[b, blk * 128:(blk + 1) * 128, :] = res.results[r]["y"][tb * 128:(tb + 1) * 128]
    return y


if __name__ == "__main__":
    data = np.load("/tmp/inputs.npz")
    inputs = {k: data[k] for k in data.files}
    out = kernel(**inputs)
    exp = np.load("/tmp/expected.npy")
    err = np.linalg.norm(out - exp) / np.linalg.norm(exp)
    print("Relative error:", err)


# revision 37
# speedup vs baseline: 1.0059x; 1.0059x over previous
"""BitAttention TRN2 kernel: 8-core SPMD (DP over batch x TP over kv-heads).

Self-contained: hardcodes shapes B=2, S=2048, D=2048, H=16, KH=4.
Core r: batch b = r//4, kv-head kh = r%4, output stripe qq = r%4.

Math (forward-equivalent to the reference):
  - linear_bit = rms_norm -> per-row int8 act quant -> ternary weight quant
    -> matmul. Act-quant scale 127/max|xn| has rms self-cancel: the quantized
    ints are round(x*127/mx); rms enters only the per-token dequant scale.
  - Activations quantize straight to f16 with a +1536 offset (f16 ulp is 1 on
    [1024,2048) so the convert rounds half-to-even like jnp.round); the
    constant 1536 offset is removed inside the matmul by accumulating
    -1536*colsum(W) built from two exact f16 hi/lo matmuls.
  - Ternary weights quantize via round(clip(w*0.5/thr,-1,1)) (equivalent to
    round(tanh)), computed with the same +1536 f16 rounding trick.
  - The reference einsum sums the query-head group axis, so Q's 16 heads
    collapse to 4: group-sum the ternary w_q head blocks (ints in [-4,4]).
  - Scale folding: both 1/sqrt(HD) plus the wq/wk arctanh scales fold into
    the q rope tables (rope is linear); the wv scale cancels through the
    output rms-norm; the wo scale folds into the final dequant.
  - Attention runs transposed (S^T = K Q^T per key block) so softmaxed probs
    feed P^T directly into PV matmuls with no PE transposes; the softmax
    denominator comes from an extra all-ones column in the PV matmul.
    No max subtraction (logits are O(1) by construction).
  - Weight quantization work is sharded: batch-pair cores split w_q/w_k/w_v
    by row blocks, all 8 cores split w_o by columns; ternarized weights are
    exchanged with AllGather.
  - All tensor-engine transposes are done by DMA-transpose (f16/bf16).
  - The attention-out exchange is an AllToAll within each batch group (4
    cores), so each received slot is one kv-head's slice, concatenating
    directly into the KVD axis.
"""
import numpy as np
from contextlib import ExitStack

import concourse.bass as bass
import concourse.bacc as bacc
import concourse.mybir as mybir
import concourse.tile as tile
from concourse.bass_utils import run_bass_kernel_spmd

B, S, D = 2, 2048, 2048
H, KH = 16, 4
HD = D // H          # 128
KVD = KH * HD        # 512
NB = S // 128        # 16 token blocks
SQ = S // 4          # 512 tokens per output stripe
EPS = 1e-8
MAGIC = float(1.5 * 2 ** 23)
M16 = 1536.0
ATANH05 = 0.5493061443340549      # arctanh(0.5)
NEG = -3.4e38
F32 = mybir.dt.float32
BF16 = mybir.dt.bfloat16
F16 = mybir.dt.float16
AX = mybir.AxisListType
OP = mybir.AluOpType
AF = mybir.ActivationFunctionType

_cache = {}


def _pt_off(kb, causal):
    if causal:
        return 2048 * kb - 64 * kb * (kb - 1)
    return 2048 * kb


def build(causal: bool, local_cc: bool = False):
    nc = bacc.Bacc()
    x_d = nc.dram_tensor("x", [S, D], F32, kind="ExternalInput")
    xs_d = nc.dram_tensor("xstat", [SQ, D], F32, kind="ExternalInput")
    wq_d = nc.dram_tensor("wq", [D // 2, KVD], F32, kind="ExternalInput")   # row-shard
    wk_d = nc.dram_tensor("wk", [D // 2, HD], F32, kind="ExternalInput")
    wv_d = nc.dram_tensor("wv", [D // 2, HD], F32, kind="ExternalInput")
    wo_d = nc.dram_tensor("wo", [KVD, D // 8], F32, kind="ExternalInput")   # col-shard
    cos_d = nc.dram_tensor("cos", [128, NB * HD // 2], F32, kind="ExternalInput")
    sin_d = nc.dram_tensor("sin", [128, NB * HD // 2], F32, kind="ExternalInput")
    y_d = nc.dram_tensor("y", [SQ, D], F32, kind="ExternalOutput")
    st_in = nc.dram_tensor("st_in", [1, 4], F32)
    sc_in = nc.dram_tensor("sc_in", [4, 128, 2], F32)
    sc_out = nc.dram_tensor("sc_out", [4, 4, 128, 2], F32)
    st_out = nc.dram_tensor("st_out", [1, 4], F32, addr_space="Shared")
    wg_in = nc.dram_tensor("wg_in", [8, 128, 3 * HD], F16)
    wg_out = nc.dram_tensor("wg_out", [2, 8, 128, 3 * HD], F16)
    wob_in = nc.dram_tensor("wob_in", [4, 128, 256], BF16)
    wob_out = nc.dram_tensor("wob_out", [8, 4, 128, 256], BF16, addr_space="Shared")
    cc_in = nc.dram_tensor("cc_in", [4, 8, 128, HD], F32)
    qsel_d = nc.dram_tensor("qsel", [128, 2], F32, kind="ExternalInput")
    cc_out = nc.dram_tensor("cc_out", [4, 8, 128, HD], F32)

    PTW = _pt_off(NB, causal)

    with tile.TileContext(nc) as tc, ExitStack() as ctx:
        cpool = ctx.enter_context(tc.tile_pool(name="const", bufs=1))
        sm = ctx.enter_context(tc.tile_pool(name="sm", bufs=1))
        wres = ctx.enter_context(tc.tile_pool(name="wres", bufs=1))

        # ---------- constants ----------
        cmT = cpool.tile([128, 128], F32, tag="cmT")
        if causal:
            nc.gpsimd.memset(cmT[:], 0.0)
            nc.gpsimd.affine_select(out=cmT[:], in_=cmT[:], compare_op=OP.is_ge,
                                    fill=NEG, base=0, pattern=[[1, 128]],
                                    channel_multiplier=-1)
        ones_f16 = cpool.tile([128, 1], F16, tag="o16")
        nc.any.memset(ones_f16[:], 1.0)
        ones_bf = cpool.tile([128, 1], BF16, tag="obf")
        nc.any.memset(ones_bf[:], 1.0)
        ones_c = cpool.tile([128, 1], F32, tag="onc")
        nc.any.memset(ones_c[:], 1.0)
        ones_r = cpool.tile([1, 128], F32, tag="onr")
        nc.any.memset(ones_r[:], 1.0)
        n192 = cpool.tile([128, 128], F16, tag="n192")
        nc.any.memset(n192[:], -192.0)
        n12 = cpool.tile([128, 128], F16, tag="n12")
        nc.any.memset(n12[:], -12.0)
        inv_n = cpool.tile([128, 4], F32, tag="invn")
        for j, numel in enumerate([D * D, KVD * D, KVD * D, D * KVD]):
            nc.any.memset(inv_n[:, j:j + 1], 1.0 / numel)
        sqscr = cpool.tile([128, D], BF16, tag="sqscr")
        qsel = cpool.tile([128, 2], F32, tag="qsel")
        nc.sync.dma_start(qsel[:], qsel_d[:])
        cos_kb = cpool.tile([128, NB, HD // 2], BF16, tag="coskb")
        sin_kb = cpool.tile([128, NB, HD // 2], BF16, tag="sinkb")
        cos_qb = cpool.tile([128, NB, HD // 2], BF16, tag="cosqb")
        sin_qb = cpool.tile([128, NB, HD // 2], BF16, tag="sinqb")

        # persistent smalls
        pr = sm.tile([128, 4], F32, tag="pr")
        st_sb = sm.tile([1, 4], F32, tag="st_sb")
        st2_sb = sm.tile([1, 4], F32, tag="st2_sb")
        totals = sm.tile([128, 4], F32, tag="totals")
        s4 = sm.tile([128, 4], F32, tag="s4")
        hi4 = sm.tile([128, 4], F32, tag="hi4")
        a4 = sm.tile([128, 4], F32, tag="a4")
        aqk = sm.tile([128, 1], F32, tag="aqk")
        mxs = sm.tile([128, 4], F32, tag="mxs")
        ssqs = sm.tile([128, 4], F32, tag="ssqs")
        sx_sb = sm.tile([128, 4, 2], F32, tag="sx_sb")
        sd_all = sm.tile([128, 16, 2], F32, tag="sd_all")

        # persistent quantized weights
        wqkv = wres.tile([128, NB, 3 * HD], F16, tag="wqkv", name="wqkv")

        # x streaming pool (closed before attention) + attention-input tiles
        atl = ctx.enter_context(tc.tile_pool(name="atl", bufs=1))
        xqTp_cm = tc.tile_pool(name="xqTp", bufs=1)
        xqTp = xqTp_cm.__enter__()
        xph_cm = tc.tile_pool(name="xph", bufs=1)
        xph = xph_cm.__enter__()
        qT = atl.tile([128, NB, 128], BF16, tag="qT", name="qT")
        kT = atl.tile([128, NB, 128], BF16, tag="kT", name="kT")
        qkv_all = sm.tile([128, NB, 3 * HD], BF16, tag="qkv_all", name="qkv_all")
        vaug = sm.tile([128, NB, 132], BF16, tag="vaug", name="vaug")
        nc.any.memset(vaug[:], 1.0)
        qr = xqTp.tile([128, NB, HD], BF16, tag="qr", name="qr")
        kr = xqTp.tile([128, NB, HD], BF16, tag="kr", name="kr")
        csr = xqTp.tile([1, 3 * HD], F32, tag="csr", name="csr")
        csbc = xqTp.tile([128, 3 * HD], F32, tag="csbc", name="csbc")
        tcs = xqTp.tile([128, 3 * HD], F32, tag="tcs", name="tcs")
        hq = xqTp.tile([128, 3 * HD], F16, tag="hq", name="hq")
        lq = xqTp.tile([128, 3 * HD], F16, tag="lq", name="lq")

        qchs = [None] * 4
        xbss = [None] * 4
        xqTc = [None] * 4

        def prep_dma(ci, eng):
            xbs = []
            for ib in range(4):
                i = 4 * ci + ib
                xb = xph.tile([128, D], F32, tag="xb", bufs=3, name=f"xb{ci}_{ib}")
                eng.dma_start(xb[:], x_d[i * 128:(i + 1) * 128, :])
                xbs.append(xb)
            xbss[ci] = xbs

        def prep_quant(ci):
            qch = xph.tile([128, 4, D], F16, tag="qch", bufs=2, name=f"qch{ci}")
            qchs[ci] = qch
            for ib in range(4):
                half_quant(qch, ib, xbss[ci][ib], 4 * ci + ib)

        def prep_tp(ci, eng):
            xqTc[ci] = xqTp.tile([128, 64, 128], F16, tag="xqTc", bufs=2,
                                 name=f"xqTc{ci}")
            eng.dma_start_transpose(
                xqTc[ci][:], qchs[ci][:].rearrange("p a b -> p (a b)"))

        def xstat_load():
            for i in range(4):
                xsb = xph.tile([128, D], F32, tag="xb", bufs=3, name=f"xs{i}")
                nc.sync.dma_start(xsb[:], xs_d[i * 128:(i + 1) * 128, :])
                nc.vector.tensor_reduce(mxs[:, i:i + 1], xsb[:], axis=AX.X,
                                        op=OP.max, apply_absolute_value=True)
                nc.scalar.activation(sqscr[:], xsb[:], AF.Square,
                                     accum_out=ssqs[:, i:i + 1])

        def xstat_fin():
            mean4 = sm.tile([128, 4], F32, tag="mean4")
            nc.vector.tensor_scalar(mean4[:], ssqs[:], 1.0 / D, EPS, op0=OP.mult, op1=OP.add)
            lg4 = sm.tile([128, 4], F32, tag="lg4")
            nc.scalar.activation(lg4[:], mean4[:], AF.Ln)
            r4 = sm.tile([128, 4], F32, tag="r4")
            nc.scalar.activation(r4[:], lg4[:], AF.Exp, scale=-0.5)
            nt4 = sm.tile([128, 4], F32, tag="nt4")
            nc.vector.tensor_tensor(nt4[:], r4[:], r4[:], op=OP.mult)
            nc.vector.tensor_tensor(nt4[:], nt4[:], mean4[:], op=OP.mult)
            nc.vector.tensor_scalar(nt4[:], nt4[:], -0.5, 1.5, op0=OP.mult, op1=OP.add)
            nc.vector.tensor_tensor(r4[:], r4[:], nt4[:], op=OP.mult)
            m127 = sm.tile([128, 4], F32, tag="m127")
            nc.vector.tensor_scalar(m127[:], mxs[:], 1.0 / 127.0, None, op0=OP.mult)
            smul4 = sm.tile([128, 4], F32, tag="smul4")
            nc.vector.reciprocal(smul4[:], m127[:])
            nc.vector.tensor_tensor(nt4[:], m127[:], smul4[:], op=OP.mult)
            nc.vector.tensor_scalar(nt4[:], nt4[:], -1.0, 2.0, op0=OP.mult, op1=OP.add)
            nc.vector.tensor_tensor(smul4[:], smul4[:], nt4[:], op=OP.mult)
            deq4 = sm.tile([128, 4], F32, tag="deq4")
            nc.vector.tensor_tensor(deq4[:], mxs[:], r4[:], op=OP.mult)
            nc.vector.tensor_scalar(deq4[:], deq4[:], 1.0 / 127.0, None, op0=OP.mult)
            nc.vector.tensor_copy(sx_sb[:, :, 0], smul4[:])
            nc.vector.tensor_copy(sx_sb[:, :, 1], deq4[:])
            nc.sync.dma_start(sc_in.ap().rearrange("i p c -> p i c"), sx_sb[:])
            if local_cc:
                nc.sync.dma_start(sc_out.ap()[0], sc_in.ap())
            else:
                nc.gpsimd.collective_compute(
                    "AllGather", OP.bypass,
                    replica_groups=[[0, 1, 2, 3], [4, 5, 6, 7]],
                    ins=[sc_in.ap().opt()], outs=[sc_out.ap().opt()])
            nc.sync.dma_start(sd_all[:], sc_out.ap().rearrange("s i p c -> p (s i) c"))

        def half_quant(qch, ib, xb, i):
            hw = D // 2
            nc.vector.tensor_scalar(qch[:, ib, 0:hw], xb[:, 0:hw],
                                    sd_all[:, i, 0:1], M16, op0=OP.mult, op1=OP.add)
            nc.scalar.activation(qch[:, ib, hw:D], xb[:, hw:D], AF.Copy,
                                 bias=M16, scale=sd_all[:, i, 0:1])

        def rope_chunk(ci):
            hh = HD // 2
            cs = slice(4 * ci, 4 * ci + 4)
            for src0, cosb, sinb, dst in ((0, cos_qb, sin_qb, qr),
                                          (HD, cos_kb, sin_kb, kr)):
                ev = qkv_all[:, cs, src0:src0 + hh]
                od = qkv_all[:, cs, src0 + hh:src0 + HD]
                t1 = xph.tile([128, 4, hh], BF16, tag="t1", bufs=2)
                t2 = xph.tile([128, 4, hh], BF16, tag="t2", bufs=2)
                nc.vector.tensor_tensor(t1[:], ev, cosb[:, cs, :], op=OP.mult)
                nc.vector.tensor_tensor(t2[:], od, sinb[:, cs, :], op=OP.mult)
                nc.vector.tensor_tensor(dst[:, cs, 0:hh], t1[:], t2[:], op=OP.subtract)
                t3 = xph.tile([128, 4, hh], BF16, tag="t1", bufs=2)
                t4 = xph.tile([128, 4, hh], BF16, tag="t2", bufs=2)
                nc.vector.tensor_tensor(t3[:], ev, sinb[:, cs, :], op=OP.mult)
                nc.vector.tensor_tensor(t4[:], od, cosb[:, cs, :], op=OP.mult)
                nc.vector.tensor_tensor(dst[:, cs, hh:HD], t3[:], t4[:], op=OP.add)
            nc.sync.dma_start_transpose(
                qT[:, cs, :], qr[:, cs, :].rearrange("p a b -> p (a b)"))
            nc.sync.dma_start_transpose(
                kT[:, cs, :], kr[:, cs, :].rearrange("p a b -> p (a b)"))

        with tc.tile_pool(name="cstage", bufs=1) as cstage:
          with tc.tile_pool(name="wph", bufs=1) as wph:
            # ---- all input DMAs up front (SP in readiness order) ----
            cosf = cstage.tile([128, NB, HD // 2], F32, tag="cosf")
            sinf = cstage.tile([128, NB, HD // 2], F32, tag="sinf")
            wq_sb = wph.tile([128, 8, KVD], F32, tag="wq_sb")
            wk_sb = wph.tile([128, 8, HD], F32, tag="wk_sb")
            wv_sb = wph.tile([128, 8, HD], F32, tag="wv_sb")
            wo_sb = wph.tile([128, 4, 256], F32, tag="wo_sb")
            nc.sync.dma_start(wq_sb[:], wq_d.ap().rearrange("(j p) c -> p j c", p=128))
            nc.sync.dma_start(wk_sb[:], wk_d.ap().rearrange("(j p) c -> p j c", p=128))
            nc.sync.dma_start(wv_sb[:], wv_d.ap().rearrange("(j p) c -> p j c", p=128))
            nc.sync.dma_start(wo_sb[:], wo_d.ap().rearrange("(c p) d -> p c d", p=128))
            # ---- pass1 |w| row sums (before x stats: heads the longer path) ----
            nc.vector.tensor_reduce(pr[:, 0:1], wq_sb[:].rearrange("p a b -> p (a b)"),
                                    axis=AX.X, op=OP.add, apply_absolute_value=True)
            nc.vector.tensor_reduce(pr[:, 1:2], wk_sb[:].rearrange("p a b -> p (a b)"),
                                    axis=AX.X, op=OP.add, apply_absolute_value=True)
            nc.vector.tensor_reduce(pr[:, 2:3], wv_sb[:].rearrange("p a b -> p (a b)"),
                                    axis=AX.X, op=OP.add, apply_absolute_value=True)
            nc.vector.tensor_reduce(pr[:, 3:4], wo_sb[:].rearrange("p a b -> p (a b)"),
                                    axis=AX.X, op=OP.add, apply_absolute_value=True)

            xstat_load()

            # ---- weight stats reduce + exchange (SP) ----
            with tc.tile_pool(name="psst", bufs=2, space="PSUM") as psst:
                pcol = psst.tile([1, 4], F32, tag="st")
                nc.tensor.matmul(pcol[:], ones_c[:], pr[:], start=True, stop=True)
                nc.vector.tensor_copy(st_sb[:], pcol[:])
                nc.sync.dma_start(st_in[:], st_sb[:])
                if local_cc:
                    nc.sync.dma_start(st_out.ap(), st_in.ap())
                else:
                    nc.gpsimd.collective_compute(
                        "AllReduce", OP.add, replica_groups=[list(range(8))],
                        ins=[st_in.ap().opt()], outs=[st_out.ap().opt()])
                nc.sync.dma_start(st2_sb[:], st_out[:])
                bc = psst.tile([128, 4], F32, tag="st")
                nc.tensor.matmul(bc[:], ones_r[:], st2_sb[:], start=True, stop=True)
                nc.vector.tensor_copy(totals[:], bc[:])

            xbs = []
            for ib in range(2):
                xb = xph.tile([128, D], F32, tag="xb", bufs=3, name=f"xb0_{ib}")
                nc.sync.dma_start(xb[:], x_d[ib * 128:(ib + 1) * 128, :])
                xbs.append(xb)
            xstat_fin()
            for ib in range(2, 4):
                xb = xph.tile([128, D], F32, tag="xb", bufs=3, name=f"xb0_{ib}")
                nc.sync.dma_start(xb[:], x_d[ib * 128:(ib + 1) * 128, :])
                xbs.append(xb)
            xbss[0] = xbs
            prep_quant(0)
            prep_dma(1, nc.sync)
            prep_quant(1)

            # ---- ternary thresholds and scales ----
            nc.vector.tensor_tensor(s4[:], totals[:], inv_n[:], op=OP.mult)
            thr2 = sm.tile([128, 4], F32, tag="thr2")
            nc.vector.tensor_scalar(thr2[:], s4[:], EPS, 2.0 * ATANH05,
                                    op0=OP.add, op1=OP.mult)
            nc.vector.reciprocal(hi4[:], thr2[:])
            ntp = sm.tile([128, 4], F32, tag="ntp")
            nc.vector.tensor_tensor(ntp[:], thr2[:], hi4[:], op=OP.mult)
            nc.vector.tensor_scalar(ntp[:], ntp[:], -1.0, 2.0, op0=OP.mult, op1=OP.add)
            nc.vector.tensor_tensor(hi4[:], hi4[:], ntp[:], op=OP.mult)
            num = sm.tile([128, 4], F32, tag="num")
            den = sm.tile([128, 4], F32, tag="den")
            rat = sm.tile([128, 4], F32, tag="rat")
            nc.vector.tensor_scalar(num[:], s4[:], 1.0, None, op0=OP.add)
            nc.vector.tensor_scalar(den[:], s4[:], -1.0, 1.0, op0=OP.mult, op1=OP.add)
            nc.vector.reciprocal(rat[:], den[:])
            nc.vector.tensor_tensor(ntp[:], den[:], rat[:], op=OP.mult)
            nc.vector.tensor_scalar(ntp[:], ntp[:], -1.0, 2.0, op0=OP.mult, op1=OP.add)
            nc.vector.tensor_tensor(rat[:], rat[:], ntp[:], op=OP.mult)
            nc.vector.tensor_tensor(rat[:], rat[:], num[:], op=OP.mult)
            lnr = sm.tile([128, 4], F32, tag="lnr")
            nc.scalar.activation(lnr[:], rat[:], AF.Ln)
            nc.vector.tensor_scalar(a4[:], lnr[:], 0.5, None, op0=OP.mult)
            nc.vector.tensor_tensor(aqk[:], a4[:, 0:1], a4[:, 1:2], op=OP.mult)
            nc.vector.tensor_scalar(aqk[:], aqk[:], 1.0 / HD, None, op0=OP.mult)

            # ---- ternary quantize shards (wq on DVE, rest on pool) ----
            wsh = wph.tile([128, 8, 3 * HD], F16, tag="wsh")
            wosh = wph.tile([128, 4, 256], BF16, tag="wosh")
            tq = wph.tile([128, 4, KVD], F16, tag="tq")
            tb16 = wph.tile([128, 1024], F16, tag="tb16")
            nc.vector.tensor_scalar(wq_sb[:], wq_sb[:], hi4[:, 0:1], 1.0,
                                    op0=OP.mult, op1=OP.min)
            for h in range(2):
                js = slice(4 * h, 4 * h + 4)
                nc.vector.tensor_scalar(tq[:], wq_sb[:, js, :], -1.0, M16,
                                        op0=OP.max, op1=OP.add)
                nc.vector.tensor_scalar(tq[:], tq[:], M16, None, op0=OP.subtract)
                nc.vector.tensor_tensor(wsh[:, js, 0:HD], tq[:, :, 0:HD],
                                        tq[:, :, HD:2 * HD], op=OP.add)
                nc.vector.scalar_tensor_tensor(wsh[:, js, 0:HD], tq[:, :, 2 * HD:3 * HD],
                                               1.0, wsh[:, js, 0:HD],
                                               op0=OP.mult, op1=OP.add)
                nc.vector.scalar_tensor_tensor(wsh[:, js, 0:HD], tq[:, :, 3 * HD:4 * HD],
                                               1.0, wsh[:, js, 0:HD],
                                               op0=OP.mult, op1=OP.add)
            tkv = tb16[:].rearrange("p (a b) -> p a b", a=8)
            nc.gpsimd.tensor_scalar(wk_sb[:], wk_sb[:], hi4[:, 1:2], 1.0,
                                    op0=OP.mult, op1=OP.min)
            nc.gpsimd.tensor_scalar(tkv, wk_sb[:], -1.0, M16, op0=OP.max, op1=OP.add)
            nc.gpsimd.tensor_scalar(wsh[:, :, HD:2 * HD], tkv, M16, None, op0=OP.subtract)
            nc.gpsimd.tensor_scalar(wv_sb[:], wv_sb[:], hi4[:, 2:3], 1.0,
                                    op0=OP.mult, op1=OP.min)
            nc.gpsimd.tensor_scalar(tkv, wv_sb[:], -1.0, M16, op0=OP.max, op1=OP.add)
            nc.gpsimd.tensor_scalar(wsh[:, :, 2 * HD:3 * HD], tkv, M16, None, op0=OP.subtract)
            two = tb16[:].rearrange("p (a b) -> p a b", a=4)
            nc.gpsimd.tensor_scalar(wo_sb[:], wo_sb[:], hi4[:, 3:4], 1.0,
                                    op0=OP.mult, op1=OP.min)
            nc.gpsimd.tensor_scalar(two, wo_sb[:], -1.0, M16, op0=OP.max, op1=OP.add)
            nc.gpsimd.tensor_scalar(wosh[:], two, M16, None, op0=OP.subtract)

            # ---- exchange ternary shards ----
            nc.sync.dma_start(wg_in.ap().rearrange("j p c -> p j c"), wsh[:])
            nc.sync.dma_start(wob_in.ap().rearrange("c p d -> p c d"), wosh[:])
            if local_cc:
                nc.sync.dma_start(wg_out.ap()[0], wg_in.ap())
            else:
                nc.gpsimd.collective_compute(
                    "AllGather", OP.bypass,
                    replica_groups=[[0, 4], [1, 5], [2, 6], [3, 7]],
                    ins=[wg_in.ap().opt()], outs=[wg_out.ap().opt()])
            for sgi in range(2):
                nc.sync.dma_start(
                    wqkv[:].rearrange("p (j s) c -> p j s c", s=2)[:, :, sgi, :],
                    wg_out.ap()[sgi].rearrange("j p c -> p j c"))
            prep_tp(0, nc.sync)
            prep_tp(1, nc.sync)
            nc.sync.dma_start(cosf[:], cos_d.ap().rearrange("p (i f) -> p i f", i=NB))
            nc.sync.dma_start(sinf[:], sin_d.ap().rearrange("p (i f) -> p i f", i=NB))
            nc.vector.tensor_copy(cos_kb[:], cosf[:])
            nc.vector.tensor_copy(sin_kb[:], sinf[:])
            nc.vector.tensor_scalar(cos_qb[:], cosf[:], aqk[:], None, op0=OP.mult)
            nc.vector.tensor_scalar(sin_qb[:], sinf[:], aqk[:], None, op0=OP.mult)
          # (wph closed: f32 weight shards freed)

        # ---------- x quantize/transpose fused with QKV ----------
        with tc.tile_pool(name="psc", bufs=3, space="PSUM") as psc:
            prep_dma(2, nc.sync)
            prep_quant(2)
            prep_tp(2, nc.sync)

            # csum of wqkv for the -1536 offset correction (f16 hi/lo split)
            csp = psc.tile([1, 3 * HD], F32, tag="cs", bufs=2)
            for j in range(NB):
                nc.tensor.matmul(csp[:], ones_f16[:], wqkv[:, j, :],
                                 start=(j == 0), stop=(j == NB - 1))
            nc.scalar.activation(csr[:], csp[:], AF.Copy)
            bcp = psc.tile([128, 3 * HD], F32, tag="cs", bufs=2)
            nc.tensor.matmul(bcp[:], ones_r[:], csr[:], start=True, stop=True)
            nc.scalar.activation(csbc[:], bcp[:], AF.Copy)
            nc.gpsimd.tensor_scalar(tcs[:], csbc[:], 1.0 / 16.0, MAGIC,
                                    op0=OP.mult, op1=OP.add)
            nc.gpsimd.tensor_scalar(tcs[:], tcs[:], MAGIC, None, op0=OP.subtract)
            nc.vector.tensor_copy(hq[:], tcs[:])
            nc.vector.scalar_tensor_tensor(lq[:], tcs[:], -16.0, csbc[:],
                                           op0=OP.mult, op1=OP.add)

            def qkv_chunk(ci):
                for ib in range(4):
                    i = 4 * ci + ib
                    pq = psc.tile([128, 3 * HD], F32, tag="mm", bufs=3)
                    for j in range(NB):
                        nc.tensor.matmul(pq[:], xqTc[ci][:, 16 * ib + j, :],
                                         wqkv[:, j, :], start=(j == 0), stop=False)
                    nc.tensor.matmul(pq[:], n192[:], hq[:], start=False, stop=False,
                                     skip_group_check=True)
                    nc.tensor.matmul(pq[:], n12[:], lq[:], start=False, stop=True,
                                     skip_group_check=True)
                    nc.vector.tensor_scalar(qkv_all[:, i, 0:2 * HD], pq[:, 0:2 * HD],
                                            sd_all[:, i, 1:2], None, op0=OP.mult)
                    nc.vector.tensor_scalar(vaug[:, i, 0:HD], pq[:, 2 * HD:3 * HD],
                                            sd_all[:, i, 1:2], None, op0=OP.mult)

            prep_dma(3, nc.sync)
            prep_quant(3)
            prep_tp(3, nc.sync)
            qkv_chunk(0)
            rope_chunk(0)
            qkv_chunk(1)
            rope_chunk(1)
            qkv_chunk(2)
            rope_chunk(2)
            qkv_chunk(3)
            rope_chunk(3)

        xph_cm.__exit__(None, None, None)
        xqTp_cm.__exit__(None, None, None)

        # ---------- attention + output projection ----------
        qTf = qT[:].rearrange("p a b -> p (a b)")
        with tc.tile_pool(name="attn", bufs=1) as attn, \
             tc.tile_pool(name="pss", bufs=3, space="PSUM") as pss, \
             tc.tile_pool(name="psv", bufs=3, space="PSUM") as psv, \
             tc.tile_pool(name="psy", bufs=2, space="PSUM") as psy:
            PT = attn.tile([128, PTW], BF16, tag="PT")
            wo_q = attn.tile([128, 4, D], BF16, tag="wo_q", name="wo_q")
            if local_cc:
                nc.scalar.dma_start(wob_out.ap()[0], wob_in.ap())
            else:
                nc.gpsimd.collective_compute(
                    "AllGather", OP.bypass, replica_groups=[list(range(8))],
                    ins=[wob_in.ap().opt()], outs=[wob_out.ap().opt()])
            for sgi in range(8):
                nc.scalar.dma_start(
                    wo_q[:].rearrange("p c (s d) -> p c s d", s=8)[:, :, sgi, :],
                    wob_out.ap()[sgi].rearrange("c p d -> p c d"))

            def scores(kb):
                qlo = 128 * kb if causal else 0
                c0 = qlo
                first = True
                while c0 < S:
                    cw = min(512, S - c0)
                    sp = pss.tile([128, 512], F32, tag="sc")
                    nc.tensor.matmul(sp[:, 0:cw], kT[:, kb, :], qTf[:, c0:c0 + cw],
                                     start=True, stop=True)
                    if causal and first:
                        nc.vector.tensor_tensor(sp[:, 0:128], sp[:, 0:128], cmT[:],
                                                op=OP.add)
                    nc.scalar.activation(PT[:, _pt_off(kb, causal) + c0 - qlo:
                                            _pt_off(kb, causal) + c0 - qlo + cw],
                                         sp[:, 0:cw], AF.Exp)
                    first = False
                    c0 += cw

            obufs = [None] * 4

            def pv(qb):
                if qb % 4 == 0:
                    obufs[qb // 4] = attn.tile([128, 4, HD], F32, tag="obuf",
                                               bufs=2, name=f"ob{qb // 4}")
                po = psv.tile([128, 132], F32, tag="po")
                nkb = qb + 1 if causal else NB
                for k2 in range(nkb):
                    qoff = (qb - k2) * 128 if causal else qb * 128
                    lhs = PT[:, _pt_off(k2, causal) + qoff:
                             _pt_off(k2, causal) + qoff + 128]
                    nc.tensor.matmul(po[:, 0:HD + 1], lhs, vaug[:, k2, 0:HD + 1],
                                     start=(k2 == 0), stop=(k2 == nkb - 1))
                rz = attn.tile([128, 1], F32, tag="rz", bufs=3)
                nz = attn.tile([128, 1], F32, tag="nz", bufs=3)
                nc.vector.reciprocal(rz[:], po[:, HD:HD + 1])
                nc.vector.tensor_tensor(nz[:], po[:, HD:HD + 1], rz[:], op=OP.mult)
                nc.vector.tensor_scalar(nz[:], nz[:], -1.0, 2.0, op0=OP.mult, op1=OP.add)
                nc.vector.tensor_tensor(rz[:], rz[:], nz[:], op=OP.mult)
                nc.vector.tensor_scalar(obufs[qb // 4][:, qb % 4, :], po[:, 0:HD],
                                        rz[:], None, op0=OP.mult)

            xos = [None] * 4

            def cc_ex(tb):
                nc.sync.dma_start(cc_in.ap()[tb, 0:4].rearrange("s p d -> p s d"),
                                  obufs[tb][:])
                nc.sync.dma_start(cc_in.ap()[tb, 4:8].rearrange("s p d -> p s d"),
                                  obufs[tb][:])
                if local_cc:
                    nc.sync.dma_start(cc_out.ap()[tb], cc_in.ap()[tb])
                else:
                    nc.gpsimd.collective_compute(
                        "AllToAll", OP.bypass, replica_groups=[list(range(8))],
                        ins=[cc_in.ap()[tb].opt()], outs=[cc_out.ap()[tb].opt()])
                xo8 = attn.tile([128, 8, HD], F32, tag="xo8", bufs=2, name=f"xo8_{tb}")
                nc.sync.dma_start(xo8[:], cc_out.ap()[tb].rearrange("s p d -> p s d"))
                xos[tb] = xo8

            def oproj(tb):
                xo8 = xos[tb]
                xsel = attn.tile([128, KVD], F32, tag="xsel", bufs=2)
                nc.gpsimd.tensor_scalar(xsel[:], xo8[:, 0:4, :].rearrange("p a b -> p (a b)"),
                                        qsel[:, 0:1], None, op0=OP.mult)
                xo = attn.tile([128, KVD], F32, tag="xo", bufs=2, name=f"xo{tb}")
                nc.vector.scalar_tensor_tensor(xo[:], xo8[:, 4:8, :].rearrange("p a b -> p (a b)"),
                                               qsel[:, 1:2], xsel[:],
                                               op0=OP.mult, op1=OP.add)
                mx2 = attn.tile([128, 1], F32, tag="mx2", bufs=2)
                nc.vector.tensor_reduce(mx2[:], xo[:], axis=AX.X, op=OP.max,
                                        apply_absolute_value=True)
                ssq2 = attn.tile([128, 1], F32, tag="ssq2", bufs=2)
                nc.scalar.activation(sqscr[:, 0:KVD], xo[:], AF.Square, accum_out=ssq2[:])
                mean2 = attn.tile([128, 1], F32, tag="mean2", bufs=2)
                nc.vector.tensor_scalar(mean2[:], ssq2[:], 1.0 / KVD, EPS,
                                        op0=OP.mult, op1=OP.add)
                lg2 = attn.tile([128, 1], F32, tag="lg2", bufs=2)
                nc.scalar.activation(lg2[:], mean2[:], AF.Ln)
                r2 = attn.tile([128, 1], F32, tag="r2", bufs=2)
                nc.scalar.activation(r2[:], lg2[:], AF.Exp, scale=-0.5)
                nt2 = attn.tile([128, 1], F32, tag="nt2", bufs=2)
                nc.vector.tensor_tensor(nt2[:], r2[:], r2[:], op=OP.mult)
                nc.vector.tensor_tensor(nt2[:], nt2[:], mean2[:], op=OP.mult)
                nc.vector.tensor_scalar(nt2[:], nt2[:], -0.5, 1.5, op0=OP.mult, op1=OP.add)
                nc.vector.tensor_tensor(r2[:], r2[:], nt2[:], op=OP.mult)
                m2 = attn.tile([128, 1], F32, tag="m2", bufs=2)
                nc.vector.tensor_scalar(m2[:], mx2[:], 1.0 / 127.0, None, op0=OP.mult)
                sl2 = attn.tile([128, 1], F32, tag="sl2", bufs=2)
                nc.vector.reciprocal(sl2[:], m2[:])
                nc.vector.tensor_tensor(nt2[:], m2[:], sl2[:], op=OP.mult)
                nc.vector.tensor_scalar(nt2[:], nt2[:], -1.0, 2.0, op0=OP.mult, op1=OP.add)
                nc.vector.tensor_tensor(sl2[:], sl2[:], nt2[:], op=OP.mult)
                dqy = attn.tile([128, 1], F32, tag="dqy", bufs=2)
                nc.vector.tensor_tensor(dqy[:], mx2[:], r2[:], op=OP.mult)
                nc.vector.tensor_scalar(dqy[:], dqy[:], 1.0 / 127.0, None, op0=OP.mult)
                nc.vector.tensor_tensor(dqy[:], dqy[:], a4[:, 3:4], op=OP.mult)
                nc.vector.tensor_scalar(xo[:], xo[:], sl2[:], MAGIC,
                                        op0=OP.mult, op1=OP.add)
                qo = attn.tile([128, KVD], BF16, tag="qo", bufs=2)
                nc.scalar.activation(qo[:], xo[:], AF.Copy, bias=-MAGIC)
                xoT = attn.tile([128, 4, 128], BF16, tag="xoT", bufs=2)
                nc.scalar.dma_start_transpose(xoT[:], qo[:])
                y_sb = attn.tile([128, D], F32, tag="ysb", bufs=2)
                for oc in range(4):
                    py = psy.tile([128, 512], F32, tag="my")
                    for jc in range(4):
                        nc.tensor.matmul(py[:], xoT[:, jc, :],
                                         wo_q[:, jc, oc * 512:(oc + 1) * 512],
                                         start=(jc == 0), stop=(jc == 3))
                    if oc % 2 == 0:
                        nc.scalar.activation(y_sb[:, oc * 512:(oc + 1) * 512], py[:],
                                             AF.Copy, scale=dqy[:])
                    else:
                        nc.vector.tensor_scalar(y_sb[:, oc * 512:(oc + 1) * 512],
                                                py[:], dqy[:], None, op0=OP.mult)
                nc.sync.dma_start(y_d[tb * 128:(tb + 1) * 128, :], y_sb[:])

            def post_pv(qb):
                # at quarter completion: launch the exchange; run the PREVIOUS
                # quarter's output projection (its data has long arrived)
                if qb % 4 == 3:
                    tb = qb // 4
                    cc_ex(tb)
                    if tb >= 1:
                        oproj(tb - 1)

            if causal:
                scores(0)
                for kb in range(1, NB):
                    scores(kb)
                    pv(kb - 1)
                    post_pv(kb - 1)
                pv(NB - 1)
                post_pv(NB - 1)
            else:
                for kb in range(NB):
                    scores(kb)
                for qb in range(NB):
                    pv(qb)
                    post_pv(qb)
            oproj(3)
    nc.compile()
    return nc


def _rope_perm():
    p = np.empty(HD, np.int64)
    p[:HD // 2] = np.arange(0, HD, 2)
    p[HD // 2:] = np.arange(1, HD, 2)
    return p


def qsel_host(b):
    q = np.zeros((128, 2), np.float32)
    q[:, b] = 1.0
    return q


def _prep_inputs(inputs):
    x = np.ascontiguousarray(np.asarray(inputs["x"], np.float32))
    w_q = np.asarray(inputs["w_q"], np.float32)
    w_k = np.asarray(inputs["w_k"], np.float32)
    w_v = np.asarray(inputs["w_v"], np.float32)
    w_o = np.asarray(inputs["w_o"], np.float32)
    cos = np.asarray(inputs["freq_cos"], np.float32)
    sin = np.asarray(inputs["freq_sin"], np.float32)
    # token-block layout: row p, cols (i, f) = table[i*128 + p, f]
    cos_ar = np.ascontiguousarray(
        cos.reshape(NB, 128, HD // 2).transpose(1, 0, 2).reshape(128, -1))
    sin_ar = np.ascontiguousarray(
        sin.reshape(NB, 128, HD // 2).transpose(1, 0, 2).reshape(128, -1))
    perm = _rope_perm()
    woT = np.ascontiguousarray(w_o.T)                      # [KVD, D]
    in_maps = []
    jrows = np.arange(D) // 128 % 2
    for r in range(8):
        b, kh = r // 4, r % 4
        heads = [g * KH + kh for g in range(4)]
        wq_sel = w_q.reshape(H, HD, D)[heads][:, perm, :]  # [4,128,D]
        wqT = np.ascontiguousarray(wq_sel.reshape(4 * HD, D).T)   # [D, 512]
        wkT = np.ascontiguousarray(w_k[kh * HD:(kh + 1) * HD][perm].T)  # [D,128]
        wvT = np.ascontiguousarray(w_v[kh * HD:(kh + 1) * HD].T)        # [D,128]
        sel = jrows == b
        in_maps.append({
            "x": x[b],
            "xstat": np.ascontiguousarray(x[b][kh * SQ:(kh + 1) * SQ]),
            "wq": np.ascontiguousarray(wqT[sel]),
            "wk": np.ascontiguousarray(wkT[sel]),
            "wv": np.ascontiguousarray(wvT[sel]),
            "wo": np.ascontiguousarray(woT[:, r * 256:(r + 1) * 256]),
            "cos": cos_ar, "sin": sin_ar,
            "qsel": qsel_host(b),
        })
    return in_maps


def _gains_trivial(inputs):
    return all(np.all(np.asarray(inputs[g]) == 1.0)
               for g in ("g_q", "g_k", "g_v", "g_o"))


def _numpy_fallback(inputs):
    """Faithful numpy reimplementation (slow); used only for unexpected configs."""
    x = np.asarray(inputs["x"], np.float32)
    cos, sin = (np.asarray(inputs[k], np.float32) for k in ("freq_cos", "freq_sin"))
    causal = int(np.asarray(inputs["causal"]))

    def rms(t, g):
        n = t * (1.0 / np.sqrt(np.mean(t * t, -1, keepdims=True, dtype=np.float32) + EPS))
        return (g * n).astype(np.float32)

    def actq(t):
        scale = 127.0 / np.clip(np.max(np.abs(t), -1, keepdims=True), 1e-4, None)
        q = np.round(t * scale)
        return np.clip(q, -128, 127) / scale

    def ternq(w):
        s = np.mean(np.abs(w), dtype=np.float32)
        return np.round(np.tanh(w / (s + EPS))) * np.arctanh(s)

    def lin(t, w, g):
        return actq(rms(t, g)).astype(np.float32) @ ternq(np.asarray(w, np.float32)).T

    Bb, Ss, Dd = x.shape
    q = lin(x, inputs["w_q"], np.asarray(inputs["g_q"], np.float32)).reshape(Bb, Ss, H, HD)
    k = lin(x, inputs["w_k"], np.asarray(inputs["g_k"], np.float32)).reshape(Bb, Ss, KH, HD)
    v = lin(x, inputs["w_v"], np.asarray(inputs["g_v"], np.float32)).reshape(Bb, Ss, KH, HD)

    def rope(t):
        t2 = t.reshape(*t.shape[:-1], -1, 2)
        c = cos[None, :, None, :]
        s_ = sin[None, :, None, :]
        o0 = t2[..., 0] * c - t2[..., 1] * s_
        o1 = t2[..., 0] * s_ + t2[..., 1] * c
        return np.stack([o0, o1], -1).reshape(t.shape).astype(np.float32)

    q, k = rope(q), rope(k)
    scale = np.float32(HD ** 0.5)
    q = q.transpose(0, 2, 1, 3) / scale
    k = k.transpose(0, 2, 1, 3)
    v = v.transpose(0, 2, 1, 3)
    qg = q.reshape(Bb, 4, KH, Ss, HD).sum(1)
    sc = np.einsum("bhnd,bhsd->bhns", qg, k).astype(np.float32)
    if causal:
        mask = np.tril(np.ones((Ss, Ss), bool))
        sc = np.where(mask[None, None], sc, np.float32(np.finfo(np.float32).min))
    sc = sc / scale
    sc = sc - sc.max(-1, keepdims=True)
    p = np.exp(sc)
    p /= p.sum(-1, keepdims=True)
    out = np.einsum("bhns,bhsd->bnhd", p, v).reshape(Bb, Ss, KVD)
    return lin(out, inputs["w_o"], np.asarray(inputs["g_o"], np.float32))


def kernel(**inputs):
    x = np.asarray(inputs["x"])
    if x.shape != (B, S, D) or not _gains_trivial(inputs):
        return _numpy_fallback(inputs)
    causal = bool(int(np.asarray(inputs["causal"])))
    key = ("bitattn", causal)
    if key not in _cache:
        _cache[key] = build(causal)
    nc = _cache[key]
    in_maps = _prep_inputs(inputs)
    res = run_bass_kernel_spmd(nc, in_maps, core_ids=list(range(8)))
    y = np.empty((B, S, D), np.float32)
    for r in range(8):
        b, qq = r // 4, r % 4
        for tb in range(4):
            blk = 4 * tb + qq
            y[b, blk * 128:(blk + 1) * 128, :] = res.results[r]["y"][tb * 128:(tb + 1) * 128]
    return y


if __name__ == "__main__":
    data = np.load("/tmp/inputs.npz")
    inputs = {k: data[k] for k in data.files}
    out = kernel(**inputs)
    exp = np.load("/tmp/expected.npy")
    err = np.linalg.norm(out - exp) / np.linalg.norm(exp)
    print("Relative error:", err)


# revision 38
# speedup vs baseline: 1.0103x; 1.0043x over previous
"""BitAttention TRN2 kernel: 8-core SPMD (DP over batch x TP over kv-heads).

Self-contained: hardcodes shapes B=2, S=2048, D=2048, H=16, KH=4.
Core r: batch b = r//4, kv-head kh = r%4, output stripe qq = r%4.

Math (forward-equivalent to the reference):
  - linear_bit = rms_norm -> per-row int8 act quant -> ternary weight quant
    -> matmul. Act-quant scale 127/max|xn| has rms self-cancel: the quantized
    ints are round(x*127/mx); rms enters only the per-token dequant scale.
  - Activations quantize straight to f16 with a +1536 offset (f16 ulp is 1 on
    [1024,2048) so the convert rounds half-to-even like jnp.round); the
    constant 1536 offset is removed inside the matmul by accumulating
    -1536*colsum(W) built from two exact f16 hi/lo matmuls.
  - Ternary weights quantize via round(clip(w*0.5/thr,-1,1)) (equivalent to
    round(tanh)), computed with the same +1536 f16 rounding trick.
  - The reference einsum sums the query-head group axis, so Q's 16 heads
    collapse to 4: group-sum the ternary w_q head blocks (ints in [-4,4]).
  - Scale folding: both 1/sqrt(HD) plus the wq/wk arctanh scales fold into
    the q rope tables (rope is linear); the wv scale cancels through the
    output rms-norm; the wo scale folds into the final dequant.
  - Attention runs transposed (S^T = K Q^T per key block) so softmaxed probs
    feed P^T directly into PV matmuls with no PE transposes; the softmax
    denominator comes from an extra all-ones column in the PV matmul.
    No max subtraction (logits are O(1) by construction).
  - Weight quantization work is sharded: batch-pair cores split w_q/w_k/w_v
    by row blocks, all 8 cores split w_o by columns; ternarized weights are
    exchanged with AllGather.
  - All tensor-engine transposes are done by DMA-transpose (f16/bf16).
  - The attention-out exchange is an AllToAll within each batch group (4
    cores), so each received slot is one kv-head's slice, concatenating
    directly into the KVD axis.
"""
import numpy as np
from contextlib import ExitStack

import concourse.bass as bass
import concourse.bacc as bacc
import concourse.mybir as mybir
import concourse.tile as tile
from concourse.bass_utils import run_bass_kernel_spmd

B, S, D = 2, 2048, 2048
H, KH = 16, 4
HD = D // H          # 128
KVD = KH * HD        # 512
NB = S // 128        # 16 token blocks
SQ = S // 4          # 512 tokens per output stripe
EPS = 1e-8
MAGIC = float(1.5 * 2 ** 23)
M16 = 1536.0
ATANH05 = 0.5493061443340549      # arctanh(0.5)
NEG = -3.4e38
F32 = mybir.dt.float32
BF16 = mybir.dt.bfloat16
F16 = mybir.dt.float16
AX = mybir.AxisListType
OP = mybir.AluOpType
AF = mybir.ActivationFunctionType

_cache = {}


def _pt_off(kb, causal):
    if causal:
        return 2048 * kb - 64 * kb * (kb - 1)
    return 2048 * kb


def build(causal: bool, local_cc: bool = False):
    nc = bacc.Bacc()
    x_d = nc.dram_tensor("x", [S, D], F32, kind="ExternalInput")
    xs_d = nc.dram_tensor("xstat", [SQ, D], F32, kind="ExternalInput")
    wq_d = nc.dram_tensor("wq", [D // 2, KVD], F32, kind="ExternalInput")   # row-shard
    wk_d = nc.dram_tensor("wk", [D // 2, HD], F32, kind="ExternalInput")
    wv_d = nc.dram_tensor("wv", [D // 2, HD], F32, kind="ExternalInput")
    wo_d = nc.dram_tensor("wo", [KVD, D // 8], F32, kind="ExternalInput")   # col-shard
    cos_d = nc.dram_tensor("cos", [128, NB * HD // 2], F32, kind="ExternalInput")
    sin_d = nc.dram_tensor("sin", [128, NB * HD // 2], F32, kind="ExternalInput")
    y_d = nc.dram_tensor("y", [SQ, D], F32, kind="ExternalOutput")
    st_in = nc.dram_tensor("st_in", [1, 4], F32)
    sc_in = nc.dram_tensor("sc_in", [4, 128, 2], F32)
    sc_out = nc.dram_tensor("sc_out", [4, 4, 128, 2], F32)
    st_out = nc.dram_tensor("st_out", [1, 4], F32, addr_space="Shared")
    wg_in = nc.dram_tensor("wg_in", [8, 128, 3 * HD], F16)
    wg_out = nc.dram_tensor("wg_out", [2, 8, 128, 3 * HD], F16)
    wob_in = nc.dram_tensor("wob_in", [4, 128, 256], BF16)
    wob_out = nc.dram_tensor("wob_out", [8, 4, 128, 256], BF16, addr_space="Shared")
    cc_in = nc.dram_tensor("cc_in", [4, 8, 128, HD], F32)
    qsel_d = nc.dram_tensor("qsel", [128, 2], F32, kind="ExternalInput")
    cc_out = nc.dram_tensor("cc_out", [4, 8, 128, HD], F32)

    PTW = _pt_off(NB, causal)

    with tile.TileContext(nc) as tc, ExitStack() as ctx:
        cpool = ctx.enter_context(tc.tile_pool(name="const", bufs=1))
        sm = ctx.enter_context(tc.tile_pool(name="sm", bufs=1))
        wres = ctx.enter_context(tc.tile_pool(name="wres", bufs=1))

        # ---------- constants ----------
        cmT = cpool.tile([128, 128], F32, tag="cmT")
        if causal:
            nc.gpsimd.memset(cmT[:], 0.0)
            nc.gpsimd.affine_select(out=cmT[:], in_=cmT[:], compare_op=OP.is_ge,
                                    fill=NEG, base=0, pattern=[[1, 128]],
                                    channel_multiplier=-1)
        ones_f16 = cpool.tile([128, 1], F16, tag="o16")
        nc.any.memset(ones_f16[:], 1.0)
        ones_bf = cpool.tile([128, 1], BF16, tag="obf")
        nc.any.memset(ones_bf[:], 1.0)
        ones_c = cpool.tile([128, 1], F32, tag="onc")
        nc.any.memset(ones_c[:], 1.0)
        ones_r = cpool.tile([1, 128], F32, tag="onr")
        nc.any.memset(ones_r[:], 1.0)
        n192 = cpool.tile([128, 128], F16, tag="n192")
        nc.any.memset(n192[:], -192.0)
        n12 = cpool.tile([128, 128], F16, tag="n12")
        nc.any.memset(n12[:], -12.0)
        inv_n = cpool.tile([128, 4], F32, tag="invn")
        for j, numel in enumerate([D * D, KVD * D, KVD * D, D * KVD]):
            nc.any.memset(inv_n[:, j:j + 1], 1.0 / numel)
        sqscr = cpool.tile([128, D], BF16, tag="sqscr")
        qsel = cpool.tile([128, 2], F32, tag="qsel")
        nc.sync.dma_start(qsel[:], qsel_d[:])
        cos_kb = cpool.tile([128, NB, HD // 2], F16, tag="coskb")
        sin_kb = cpool.tile([128, NB, HD // 2], F16, tag="sinkb")
        cos_qb = cpool.tile([128, NB, HD // 2], F16, tag="cosqb")
        sin_qb = cpool.tile([128, NB, HD // 2], F16, tag="sinqb")

        # persistent smalls
        pr = sm.tile([128, 4], F32, tag="pr")
        st_sb = sm.tile([1, 4], F32, tag="st_sb")
        st2_sb = sm.tile([1, 4], F32, tag="st2_sb")
        totals = sm.tile([128, 4], F32, tag="totals")
        s4 = sm.tile([128, 4], F32, tag="s4")
        hi4 = sm.tile([128, 4], F32, tag="hi4")
        a4 = sm.tile([128, 4], F32, tag="a4")
        aqk = sm.tile([128, 1], F32, tag="aqk")
        mxs = sm.tile([128, 4], F32, tag="mxs")
        ssqs = sm.tile([128, 4], F32, tag="ssqs")
        sx_sb = sm.tile([128, 4, 2], F32, tag="sx_sb")
        sd_all = sm.tile([128, 16, 2], F32, tag="sd_all")

        # persistent quantized weights
        wqkv = wres.tile([128, NB, 3 * HD], F16, tag="wqkv", name="wqkv")

        # x streaming pool (closed before attention) + attention-input tiles
        atl = ctx.enter_context(tc.tile_pool(name="atl", bufs=1))
        xqTp_cm = tc.tile_pool(name="xqTp", bufs=1)
        xqTp = xqTp_cm.__enter__()
        xph_cm = tc.tile_pool(name="xph", bufs=1)
        xph = xph_cm.__enter__()
        qT = atl.tile([128, NB, 128], F16, tag="qT", name="qT")
        kT = atl.tile([128, NB, 128], F16, tag="kT", name="kT")
        qkv_all = sm.tile([128, NB, 3 * HD], F16, tag="qkv_all", name="qkv_all")
        vaug = sm.tile([128, NB, 132], F16, tag="vaug", name="vaug")
        nc.any.memset(vaug[:], 1.0)
        qr = xqTp.tile([128, NB, HD], F16, tag="qr", name="qr")
        kr = xqTp.tile([128, NB, HD], F16, tag="kr", name="kr")
        csr = xqTp.tile([1, 3 * HD], F32, tag="csr", name="csr")
        csbc = xqTp.tile([128, 3 * HD], F32, tag="csbc", name="csbc")
        tcs = xqTp.tile([128, 3 * HD], F32, tag="tcs", name="tcs")
        hq = xqTp.tile([128, 3 * HD], F16, tag="hq", name="hq")
        lq = xqTp.tile([128, 3 * HD], F16, tag="lq", name="lq")

        qchs = [None] * 4
        xbss = [None] * 4
        xqTc = [None] * 4

        def prep_dma(ci, eng):
            xbs = []
            for ib in range(4):
                i = 4 * ci + ib
                xb = xph.tile([128, D], F32, tag="xb", bufs=3, name=f"xb{ci}_{ib}")
                eng.dma_start(xb[:], x_d[i * 128:(i + 1) * 128, :])
                xbs.append(xb)
            xbss[ci] = xbs

        def prep_quant(ci):
            qch = xph.tile([128, 4, D], F16, tag="qch", bufs=2, name=f"qch{ci}")
            qchs[ci] = qch
            for ib in range(4):
                half_quant(qch, ib, xbss[ci][ib], 4 * ci + ib)

        def prep_tp(ci, eng):
            xqTc[ci] = xqTp.tile([128, 64, 128], F16, tag="xqTc", bufs=2,
                                 name=f"xqTc{ci}")
            eng.dma_start_transpose(
                xqTc[ci][:], qchs[ci][:].rearrange("p a b -> p (a b)"))

        def xstat_load():
            for i in range(4):
                xsb = xph.tile([128, D], F32, tag="xb", bufs=3, name=f"xs{i}")
                nc.sync.dma_start(xsb[:], xs_d[i * 128:(i + 1) * 128, :])
                nc.vector.tensor_reduce(mxs[:, i:i + 1], xsb[:], axis=AX.X,
                                        op=OP.max, apply_absolute_value=True)
                nc.scalar.activation(sqscr[:], xsb[:], AF.Square,
                                     accum_out=ssqs[:, i:i + 1])

        def xstat_fin():
            mean4 = sm.tile([128, 4], F32, tag="mean4")
            nc.vector.tensor_scalar(mean4[:], ssqs[:], 1.0 / D, EPS, op0=OP.mult, op1=OP.add)
            lg4 = sm.tile([128, 4], F32, tag="lg4")
            nc.scalar.activation(lg4[:], mean4[:], AF.Ln)
            r4 = sm.tile([128, 4], F32, tag="r4")
            nc.scalar.activation(r4[:], lg4[:], AF.Exp, scale=-0.5)
            nt4 = sm.tile([128, 4], F32, tag="nt4")
            nc.vector.tensor_tensor(nt4[:], r4[:], r4[:], op=OP.mult)
            nc.vector.tensor_tensor(nt4[:], nt4[:], mean4[:], op=OP.mult)
            nc.vector.tensor_scalar(nt4[:], nt4[:], -0.5, 1.5, op0=OP.mult, op1=OP.add)
            nc.vector.tensor_tensor(r4[:], r4[:], nt4[:], op=OP.mult)
            m127 = sm.tile([128, 4], F32, tag="m127")
            nc.vector.tensor_scalar(m127[:], mxs[:], 1.0 / 127.0, None, op0=OP.mult)
            smul4 = sm.tile([128, 4], F32, tag="smul4")
            nc.vector.reciprocal(smul4[:], m127[:])
            nc.vector.tensor_tensor(nt4[:], m127[:], smul4[:], op=OP.mult)
            nc.vector.tensor_scalar(nt4[:], nt4[:], -1.0, 2.0, op0=OP.mult, op1=OP.add)
            nc.vector.tensor_tensor(smul4[:], smul4[:], nt4[:], op=OP.mult)
            deq4 = sm.tile([128, 4], F32, tag="deq4")
            nc.vector.tensor_tensor(deq4[:], mxs[:], r4[:], op=OP.mult)
            nc.vector.tensor_scalar(deq4[:], deq4[:], 1.0 / 127.0, None, op0=OP.mult)
            nc.vector.tensor_copy(sx_sb[:, :, 0], smul4[:])
            nc.vector.tensor_copy(sx_sb[:, :, 1], deq4[:])
            nc.sync.dma_start(sc_in.ap().rearrange("i p c -> p i c"), sx_sb[:])
            if local_cc:
                nc.sync.dma_start(sc_out.ap()[0], sc_in.ap())
            else:
                nc.gpsimd.collective_compute(
                    "AllGather", OP.bypass,
                    replica_groups=[[0, 1, 2, 3], [4, 5, 6, 7]],
                    ins=[sc_in.ap().opt()], outs=[sc_out.ap().opt()])
            nc.sync.dma_start(sd_all[:], sc_out.ap().rearrange("s i p c -> p (s i) c"))

        def half_quant(qch, ib, xb, i):
            hw = D // 2
            nc.vector.tensor_scalar(qch[:, ib, 0:hw], xb[:, 0:hw],
                                    sd_all[:, i, 0:1], M16, op0=OP.mult, op1=OP.add)
            nc.scalar.activation(qch[:, ib, hw:D], xb[:, hw:D], AF.Copy,
                                 bias=M16, scale=sd_all[:, i, 0:1])

        def rope_chunk(ci):
            hh = HD // 2
            cs = slice(4 * ci, 4 * ci + 4)
            for src0, cosb, sinb, dst in ((0, cos_qb, sin_qb, qr),
                                          (HD, cos_kb, sin_kb, kr)):
                ev = qkv_all[:, cs, src0:src0 + hh]
                od = qkv_all[:, cs, src0 + hh:src0 + HD]
                t1 = xph.tile([128, 4, hh], F16, tag="t1", bufs=2)
                t2 = xph.tile([128, 4, hh], F16, tag="t2", bufs=2)
                nc.vector.tensor_tensor(t1[:], ev, cosb[:, cs, :], op=OP.mult)
                nc.vector.tensor_tensor(t2[:], od, sinb[:, cs, :], op=OP.mult)
                nc.vector.tensor_tensor(dst[:, cs, 0:hh], t1[:], t2[:], op=OP.subtract)
                t3 = xph.tile([128, 4, hh], F16, tag="t1", bufs=2)
                t4 = xph.tile([128, 4, hh], F16, tag="t2", bufs=2)
                nc.vector.tensor_tensor(t3[:], ev, sinb[:, cs, :], op=OP.mult)
                nc.vector.tensor_tensor(t4[:], od, cosb[:, cs, :], op=OP.mult)
                nc.vector.tensor_tensor(dst[:, cs, hh:HD], t3[:], t4[:], op=OP.add)
            nc.sync.dma_start_transpose(
                qT[:, cs, :], qr[:, cs, :].rearrange("p a b -> p (a b)"))
            nc.sync.dma_start_transpose(
                kT[:, cs, :], kr[:, cs, :].rearrange("p a b -> p (a b)"))

        with tc.tile_pool(name="cstage", bufs=1) as cstage:
          with tc.tile_pool(name="wph", bufs=1) as wph:
            # ---- all input DMAs up front (SP in readiness order) ----
            cosf = cstage.tile([128, NB, HD // 2], F32, tag="cosf")
            sinf = cstage.tile([128, NB, HD // 2], F32, tag="sinf")
            wq_sb = wph.tile([128, 8, KVD], F32, tag="wq_sb")
            wk_sb = wph.tile([128, 8, HD], F32, tag="wk_sb")
            wv_sb = wph.tile([128, 8, HD], F32, tag="wv_sb")
            wo_sb = wph.tile([128, 4, 256], F32, tag="wo_sb")
            nc.sync.dma_start(wq_sb[:], wq_d.ap().rearrange("(j p) c -> p j c", p=128))
            nc.sync.dma_start(wk_sb[:], wk_d.ap().rearrange("(j p) c -> p j c", p=128))
            nc.sync.dma_start(wv_sb[:], wv_d.ap().rearrange("(j p) c -> p j c", p=128))
            nc.sync.dma_start(wo_sb[:], wo_d.ap().rearrange("(c p) d -> p c d", p=128))
            # ---- pass1 |w| row sums (before x stats: heads the longer path) ----
            nc.vector.tensor_reduce(pr[:, 0:1], wq_sb[:].rearrange("p a b -> p (a b)"),
                                    axis=AX.X, op=OP.add, apply_absolute_value=True)
            nc.vector.tensor_reduce(pr[:, 1:2], wk_sb[:].rearrange("p a b -> p (a b)"),
                                    axis=AX.X, op=OP.add, apply_absolute_value=True)
            nc.vector.tensor_reduce(pr[:, 2:3], wv_sb[:].rearrange("p a b -> p (a b)"),
                                    axis=AX.X, op=OP.add, apply_absolute_value=True)
            nc.vector.tensor_reduce(pr[:, 3:4], wo_sb[:].rearrange("p a b -> p (a b)"),
                                    axis=AX.X, op=OP.add, apply_absolute_value=True)

            xstat_load()

            # ---- weight stats reduce + exchange (SP) ----
            with tc.tile_pool(name="psst", bufs=2, space="PSUM") as psst:
                pcol = psst.tile([1, 4], F32, tag="st")
                nc.tensor.matmul(pcol[:], ones_c[:], pr[:], start=True, stop=True)
                nc.vector.tensor_copy(st_sb[:], pcol[:])
                nc.sync.dma_start(st_in[:], st_sb[:])
                if local_cc:
                    nc.sync.dma_start(st_out.ap(), st_in.ap())
                else:
                    nc.gpsimd.collective_compute(
                        "AllReduce", OP.add, replica_groups=[list(range(8))],
                        ins=[st_in.ap().opt()], outs=[st_out.ap().opt()])
                nc.sync.dma_start(st2_sb[:], st_out[:])
                bc = psst.tile([128, 4], F32, tag="st")
                nc.tensor.matmul(bc[:], ones_r[:], st2_sb[:], start=True, stop=True)
                nc.vector.tensor_copy(totals[:], bc[:])

            xbs = []
            for ib in range(2):
                xb = xph.tile([128, D], F32, tag="xb", bufs=3, name=f"xb0_{ib}")
                nc.sync.dma_start(xb[:], x_d[ib * 128:(ib + 1) * 128, :])
                xbs.append(xb)
            xstat_fin()
            for ib in range(2, 4):
                xb = xph.tile([128, D], F32, tag="xb", bufs=3, name=f"xb0_{ib}")
                nc.sync.dma_start(xb[:], x_d[ib * 128:(ib + 1) * 128, :])
                xbs.append(xb)
            xbss[0] = xbs
            prep_quant(0)
            prep_dma(1, nc.sync)
            prep_quant(1)

            # ---- ternary thresholds and scales ----
            nc.vector.tensor_tensor(s4[:], totals[:], inv_n[:], op=OP.mult)
            thr2 = sm.tile([128, 4], F32, tag="thr2")
            nc.vector.tensor_scalar(thr2[:], s4[:], EPS, 2.0 * ATANH05,
                                    op0=OP.add, op1=OP.mult)
            nc.vector.reciprocal(hi4[:], thr2[:])
            ntp = sm.tile([128, 4], F32, tag="ntp")
            nc.vector.tensor_tensor(ntp[:], thr2[:], hi4[:], op=OP.mult)
            nc.vector.tensor_scalar(ntp[:], ntp[:], -1.0, 2.0, op0=OP.mult, op1=OP.add)
            nc.vector.tensor_tensor(hi4[:], hi4[:], ntp[:], op=OP.mult)
            num = sm.tile([128, 4], F32, tag="num")
            den = sm.tile([128, 4], F32, tag="den")
            rat = sm.tile([128, 4], F32, tag="rat")
            nc.vector.tensor_scalar(num[:], s4[:], 1.0, None, op0=OP.add)
            nc.vector.tensor_scalar(den[:], s4[:], -1.0, 1.0, op0=OP.mult, op1=OP.add)
            nc.vector.reciprocal(rat[:], den[:])
            nc.vector.tensor_tensor(ntp[:], den[:], rat[:], op=OP.mult)
            nc.vector.tensor_scalar(ntp[:], ntp[:], -1.0, 2.0, op0=OP.mult, op1=OP.add)
            nc.vector.tensor_tensor(rat[:], rat[:], ntp[:], op=OP.mult)
            nc.vector.tensor_tensor(rat[:], rat[:], num[:], op=OP.mult)
            lnr = sm.tile([128, 4], F32, tag="lnr")
            nc.scalar.activation(lnr[:], rat[:], AF.Ln)
            nc.vector.tensor_scalar(a4[:], lnr[:], 0.5, None, op0=OP.mult)
            nc.vector.tensor_tensor(aqk[:], a4[:, 0:1], a4[:, 1:2], op=OP.mult)
            nc.vector.tensor_scalar(aqk[:], aqk[:], 1.0 / HD, None, op0=OP.mult)
            lga = sm.tile([128, 1], F32, tag="lga")
            nc.scalar.activation(lga[:], aqk[:], AF.Ln)
            nc.scalar.activation(aqk[:], lga[:], AF.Exp, scale=0.5)

            # ---- ternary quantize shards (wq on DVE, rest on pool) ----
            wsh = wph.tile([128, 8, 3 * HD], F16, tag="wsh")
            wosh = wph.tile([128, 4, 256], BF16, tag="wosh")
            tq = wph.tile([128, 4, KVD], F16, tag="tq")
            tb16 = wph.tile([128, 1024], F16, tag="tb16")
            nc.vector.tensor_scalar(wq_sb[:], wq_sb[:], hi4[:, 0:1], 1.0,
                                    op0=OP.mult, op1=OP.min)
            for h in range(2):
                js = slice(4 * h, 4 * h + 4)
                nc.vector.tensor_scalar(tq[:], wq_sb[:, js, :], -1.0, M16,
                                        op0=OP.max, op1=OP.add)
                nc.vector.tensor_scalar(tq[:], tq[:], M16, None, op0=OP.subtract)
                nc.vector.tensor_tensor(wsh[:, js, 0:HD], tq[:, :, 0:HD],
                                        tq[:, :, HD:2 * HD], op=OP.add)
                nc.vector.scalar_tensor_tensor(wsh[:, js, 0:HD], tq[:, :, 2 * HD:3 * HD],
                                               1.0, wsh[:, js, 0:HD],
                                               op0=OP.mult, op1=OP.add)
                nc.vector.scalar_tensor_tensor(wsh[:, js, 0:HD], tq[:, :, 3 * HD:4 * HD],
                                               1.0, wsh[:, js, 0:HD],
                                               op0=OP.mult, op1=OP.add)
            tkv = tb16[:].rearrange("p (a b) -> p a b", a=8)
            nc.gpsimd.tensor_scalar(wk_sb[:], wk_sb[:], hi4[:, 1:2], 1.0,
                                    op0=OP.mult, op1=OP.min)
            nc.gpsimd.tensor_scalar(tkv, wk_sb[:], -1.0, M16, op0=OP.max, op1=OP.add)
            nc.gpsimd.tensor_scalar(wsh[:, :, HD:2 * HD], tkv, M16, None, op0=OP.subtract)
            nc.gpsimd.tensor_scalar(wv_sb[:], wv_sb[:], hi4[:, 2:3], 1.0,
                                    op0=OP.mult, op1=OP.min)
            nc.gpsimd.tensor_scalar(tkv, wv_sb[:], -1.0, M16, op0=OP.max, op1=OP.add)
            nc.gpsimd.tensor_scalar(wsh[:, :, 2 * HD:3 * HD], tkv, M16, None, op0=OP.subtract)
            two = tb16[:].rearrange("p (a b) -> p a b", a=4)
            nc.gpsimd.tensor_scalar(wo_sb[:], wo_sb[:], hi4[:, 3:4], 1.0,
                                    op0=OP.mult, op1=OP.min)
            nc.gpsimd.tensor_scalar(two, wo_sb[:], -1.0, M16, op0=OP.max, op1=OP.add)
            nc.gpsimd.tensor_scalar(wosh[:], two, M16, None, op0=OP.subtract)

            # ---- exchange ternary shards ----
            nc.sync.dma_start(wg_in.ap().rearrange("j p c -> p j c"), wsh[:])
            nc.sync.dma_start(wob_in.ap().rearrange("c p d -> p c d"), wosh[:])
            if local_cc:
                nc.sync.dma_start(wg_out.ap()[0], wg_in.ap())
            else:
                nc.gpsimd.collective_compute(
                    "AllGather", OP.bypass,
                    replica_groups=[[0, 4], [1, 5], [2, 6], [3, 7]],
                    ins=[wg_in.ap().opt()], outs=[wg_out.ap().opt()])
            for sgi in range(2):
                nc.sync.dma_start(
                    wqkv[:].rearrange("p (j s) c -> p j s c", s=2)[:, :, sgi, :],
                    wg_out.ap()[sgi].rearrange("j p c -> p j c"))
            prep_tp(0, nc.sync)
            prep_tp(1, nc.sync)
            nc.sync.dma_start(cosf[:], cos_d.ap().rearrange("p (i f) -> p i f", i=NB))
            nc.sync.dma_start(sinf[:], sin_d.ap().rearrange("p (i f) -> p i f", i=NB))
            nc.vector.tensor_scalar(cos_kb[:], cosf[:], aqk[:], None, op0=OP.mult)
            nc.vector.tensor_scalar(sin_kb[:], sinf[:], aqk[:], None, op0=OP.mult)
            nc.vector.tensor_scalar(cos_qb[:], cosf[:], aqk[:], None, op0=OP.mult)
            nc.vector.tensor_scalar(sin_qb[:], sinf[:], aqk[:], None, op0=OP.mult)
          # (wph closed: f32 weight shards freed)

        # ---------- x quantize/transpose fused with QKV ----------
        with tc.tile_pool(name="psc", bufs=3, space="PSUM") as psc:
            prep_dma(2, nc.sync)
            prep_quant(2)
            prep_tp(2, nc.sync)

            # csum of wqkv for the -1536 offset correction (f16 hi/lo split)
            csp = psc.tile([1, 3 * HD], F32, tag="cs", bufs=2)
            for j in range(NB):
                nc.tensor.matmul(csp[:], ones_f16[:], wqkv[:, j, :],
                                 start=(j == 0), stop=(j == NB - 1))
            nc.scalar.activation(csr[:], csp[:], AF.Copy)
            bcp = psc.tile([128, 3 * HD], F32, tag="cs", bufs=2)
            nc.tensor.matmul(bcp[:], ones_r[:], csr[:], start=True, stop=True)
            nc.scalar.activation(csbc[:], bcp[:], AF.Copy)
            nc.gpsimd.tensor_scalar(tcs[:], csbc[:], 1.0 / 16.0, MAGIC,
                                    op0=OP.mult, op1=OP.add)
            nc.gpsimd.tensor_scalar(tcs[:], tcs[:], MAGIC, None, op0=OP.subtract)
            nc.vector.tensor_copy(hq[:], tcs[:])
            nc.vector.scalar_tensor_tensor(lq[:], tcs[:], -16.0, csbc[:],
                                           op0=OP.mult, op1=OP.add)

            def qkv_chunk(ci):
                for ib in range(4):
                    i = 4 * ci + ib
                    pq = psc.tile([128, 3 * HD], F32, tag="mm", bufs=3)
                    for j in range(NB):
                        nc.tensor.matmul(pq[:], xqTc[ci][:, 16 * ib + j, :],
                                         wqkv[:, j, :], start=(j == 0), stop=False)
                    nc.tensor.matmul(pq[:], n192[:], hq[:], start=False, stop=False,
                                     skip_group_check=True)
                    nc.tensor.matmul(pq[:], n12[:], lq[:], start=False, stop=True,
                                     skip_group_check=True)
                    nc.vector.tensor_scalar(qkv_all[:, i, 0:2 * HD], pq[:, 0:2 * HD],
                                            sd_all[:, i, 1:2], None, op0=OP.mult)
                    nc.vector.tensor_scalar(vaug[:, i, 0:HD], pq[:, 2 * HD:3 * HD],
                                            sd_all[:, i, 1:2], None, op0=OP.mult)

            prep_dma(3, nc.sync)
            prep_quant(3)
            prep_tp(3, nc.sync)
            qkv_chunk(0)
            rope_chunk(0)
            qkv_chunk(1)
            rope_chunk(1)
            qkv_chunk(2)
            rope_chunk(2)
            qkv_chunk(3)
            rope_chunk(3)

        xph_cm.__exit__(None, None, None)
        xqTp_cm.__exit__(None, None, None)

        # ---------- attention + output projection ----------
        qTf = qT[:].rearrange("p a b -> p (a b)")
        with tc.tile_pool(name="attn", bufs=1) as attn, \
             tc.tile_pool(name="pss", bufs=3, space="PSUM") as pss, \
             tc.tile_pool(name="psv", bufs=3, space="PSUM") as psv, \
             tc.tile_pool(name="psy", bufs=2, space="PSUM") as psy:
            PT = attn.tile([128, PTW], F16, tag="PT")
            wo_q = attn.tile([128, 4, D], BF16, tag="wo_q", name="wo_q")
            if local_cc:
                nc.scalar.dma_start(wob_out.ap()[0], wob_in.ap())
            else:
                nc.gpsimd.collective_compute(
                    "AllGather", OP.bypass, replica_groups=[list(range(8))],
                    ins=[wob_in.ap().opt()], outs=[wob_out.ap().opt()])
            for sgi in range(8):
                nc.scalar.dma_start(
                    wo_q[:].rearrange("p c (s d) -> p c s d", s=8)[:, :, sgi, :],
                    wob_out.ap()[sgi].rearrange("c p d -> p c d"))

            def scores(kb):
                qlo = 128 * kb if causal else 0
                c0 = qlo
                first = True
                while c0 < S:
                    cw = min(512, S - c0)
                    sp = pss.tile([128, 512], F32, tag="sc")
                    nc.tensor.matmul(sp[:, 0:cw], kT[:, kb, :], qTf[:, c0:c0 + cw],
                                     start=True, stop=True)
                    if causal and first:
                        nc.vector.tensor_tensor(sp[:, 0:128], sp[:, 0:128], cmT[:],
                                                op=OP.add)
                    nc.scalar.activation(PT[:, _pt_off(kb, causal) + c0 - qlo:
                                            _pt_off(kb, causal) + c0 - qlo + cw],
                                         sp[:, 0:cw], AF.Exp)
                    first = False
                    c0 += cw

            obufs = [None] * 4

            def pv(qb):
                if qb % 4 == 0:
                    obufs[qb // 4] = attn.tile([128, 4, HD], F32, tag="obuf",
                                               bufs=2, name=f"ob{qb // 4}")
                po = psv.tile([128, 132], F32, tag="po")
                nkb = qb + 1 if causal else NB
                for k2 in range(nkb):
                    qoff = (qb - k2) * 128 if causal else qb * 128
                    lhs = PT[:, _pt_off(k2, causal) + qoff:
                             _pt_off(k2, causal) + qoff + 128]
                    nc.tensor.matmul(po[:, 0:HD + 1], lhs, vaug[:, k2, 0:HD + 1],
                                     start=(k2 == 0), stop=(k2 == nkb - 1))
                rz = attn.tile([128, 1], F32, tag="rz", bufs=3)
                nz = attn.tile([128, 1], F32, tag="nz", bufs=3)
                nc.vector.reciprocal(rz[:], po[:, HD:HD + 1])
                nc.vector.tensor_tensor(nz[:], po[:, HD:HD + 1], rz[:], op=OP.mult)
                nc.vector.tensor_scalar(nz[:], nz[:], -1.0, 2.0, op0=OP.mult, op1=OP.add)
                nc.vector.tensor_tensor(rz[:], rz[:], nz[:], op=OP.mult)
                nc.vector.tensor_scalar(obufs[qb // 4][:, qb % 4, :], po[:, 0:HD],
                                        rz[:], None, op0=OP.mult)

            xos = [None] * 4

            def cc_ex(tb):
                nc.sync.dma_start(cc_in.ap()[tb, 0:4].rearrange("s p d -> p s d"),
                                  obufs[tb][:])
                nc.sync.dma_start(cc_in.ap()[tb, 4:8].rearrange("s p d -> p s d"),
                                  obufs[tb][:])
                if local_cc:
                    nc.sync.dma_start(cc_out.ap()[tb], cc_in.ap()[tb])
                else:
                    nc.gpsimd.collective_compute(
                        "AllToAll", OP.bypass, replica_groups=[list(range(8))],
                        ins=[cc_in.ap()[tb].opt()], outs=[cc_out.ap()[tb].opt()])
                xo8 = attn.tile([128, 8, HD], F32, tag="xo8", bufs=2, name=f"xo8_{tb}")
                nc.sync.dma_start(xo8[:], cc_out.ap()[tb].rearrange("s p d -> p s d"))
                xos[tb] = xo8

            def oproj(tb):
                xo8 = xos[tb]
                xsel = attn.tile([128, KVD], F32, tag="xsel", bufs=2)
                nc.gpsimd.tensor_scalar(xsel[:], xo8[:, 0:4, :].rearrange("p a b -> p (a b)"),
                                        qsel[:, 0:1], None, op0=OP.mult)
                xo = attn.tile([128, KVD], F32, tag="xo", bufs=2, name=f"xo{tb}")
                nc.vector.scalar_tensor_tensor(xo[:], xo8[:, 4:8, :].rearrange("p a b -> p (a b)"),
                                               qsel[:, 1:2], xsel[:],
                                               op0=OP.mult, op1=OP.add)
                mx2 = attn.tile([128, 1], F32, tag="mx2", bufs=2)
                nc.vector.tensor_reduce(mx2[:], xo[:], axis=AX.X, op=OP.max,
                                        apply_absolute_value=True)
                ssq2 = attn.tile([128, 1], F32, tag="ssq2", bufs=2)
                nc.scalar.activation(sqscr[:, 0:KVD], xo[:], AF.Square, accum_out=ssq2[:])
                mean2 = attn.tile([128, 1], F32, tag="mean2", bufs=2)
                nc.vector.tensor_scalar(mean2[:], ssq2[:], 1.0 / KVD, EPS,
                                        op0=OP.mult, op1=OP.add)
                lg2 = attn.tile([128, 1], F32, tag="lg2", bufs=2)
                nc.scalar.activation(lg2[:], mean2[:], AF.Ln)
                r2 = attn.tile([128, 1], F32, tag="r2", bufs=2)
                nc.scalar.activation(r2[:], lg2[:], AF.Exp, scale=-0.5)
                nt2 = attn.tile([128, 1], F32, tag="nt2", bufs=2)
                nc.vector.tensor_tensor(nt2[:], r2[:], r2[:], op=OP.mult)
                nc.vector.tensor_tensor(nt2[:], nt2[:], mean2[:], op=OP.mult)
                nc.vector.tensor_scalar(nt2[:], nt2[:], -0.5, 1.5, op0=OP.mult, op1=OP.add)
                nc.vector.tensor_tensor(r2[:], r2[:], nt2[:], op=OP.mult)
                m2 = attn.tile([128, 1], F32, tag="m2", bufs=2)
                nc.vector.tensor_scalar(m2[:], mx2[:], 1.0 / 127.0, None, op0=OP.mult)
                sl2 = attn.tile([128, 1], F32, tag="sl2", bufs=2)
                nc.vector.reciprocal(sl2[:], m2[:])
                nc.vector.tensor_tensor(nt2[:], m2[:], sl2[:], op=OP.mult)
                nc.vector.tensor_scalar(nt2[:], nt2[:], -1.0, 2.0, op0=OP.mult, op1=OP.add)
                nc.vector.tensor_tensor(sl2[:], sl2[:], nt2[:], op=OP.mult)
                dqy = attn.tile([128, 1], F32, tag="dqy", bufs=2)
                nc.vector.tensor_tensor(dqy[:], mx2[:], r2[:], op=OP.mult)
                nc.vector.tensor_scalar(dqy[:], dqy[:], 1.0 / 127.0, None, op0=OP.mult)
                nc.vector.tensor_tensor(dqy[:], dqy[:], a4[:, 3:4], op=OP.mult)
                nc.vector.tensor_scalar(xo[:], xo[:], sl2[:], MAGIC,
                                        op0=OP.mult, op1=OP.add)
                qo = attn.tile([128, KVD], BF16, tag="qo", bufs=2)
                nc.scalar.activation(qo[:], xo[:], AF.Copy, bias=-MAGIC)
                xoT = attn.tile([128, 4, 128], BF16, tag="xoT", bufs=2)
                nc.scalar.dma_start_transpose(xoT[:], qo[:])
                y_sb = attn.tile([128, D], F32, tag="ysb", bufs=2)
                for oc in range(4):
                    py = psy.tile([128, 512], F32, tag="my")
                    for jc in range(4):
                        nc.tensor.matmul(py[:], xoT[:, jc, :],
                                         wo_q[:, jc, oc * 512:(oc + 1) * 512],
                                         start=(jc == 0), stop=(jc == 3))
                    if oc % 2 == 0:
                        nc.scalar.activation(y_sb[:, oc * 512:(oc + 1) * 512], py[:],
                                             AF.Copy, scale=dqy[:])
                    else:
                        nc.vector.tensor_scalar(y_sb[:, oc * 512:(oc + 1) * 512],
                                                py[:], dqy[:], None, op0=OP.mult)
                nc.sync.dma_start(y_d[tb * 128:(tb + 1) * 128, :], y_sb[:])

            def post_pv(qb):
                # at quarter completion: launch the exchange; run the PREVIOUS
                # quarter's output projection (its data has long arrived)
                if qb % 4 == 3:
                    tb = qb // 4
                    cc_ex(tb)
                    if tb >= 1:
                        oproj(tb - 1)

            if causal:
                scores(0)
                for kb in range(1, NB):
                    scores(kb)
                    pv(kb - 1)
                    post_pv(kb - 1)
                pv(NB - 1)
                post_pv(NB - 1)
            else:
                for kb in range(NB):
                    scores(kb)
                for qb in range(NB):
                    pv(qb)
                    post_pv(qb)
            oproj(3)
    nc.compile()
    return nc


def _rope_perm():
    p = np.empty(HD, np.int64)
    p[:HD // 2] = np.arange(0, HD, 2)
    p[HD // 2:] = np.arange(1, HD, 2)
    return p


def qsel_host(b):
    q = np.zeros((128, 2), np.float32)
    q[:, b] = 1.0
    return q


def _prep_inputs(inputs):
    x = np.ascontiguousarray(np.asarray(inputs["x"], np.float32))
    w_q = np.asarray(inputs["w_q"], np.float32)
    w_k = np.asarray(inputs["w_k"], np.float32)
    w_v = np.asarray(inputs["w_v"], np.float32)
    w_o = np.asarray(inputs["w_o"], np.float32)
    cos = np.asarray(inputs["freq_cos"], np.float32)
    sin = np.asarray(inputs["freq_sin"], np.float32)
    # token-block layout: row p, cols (i, f) = table[i*128 + p, f]
    cos_ar = np.ascontiguousarray(
        cos.reshape(NB, 128, HD // 2).transpose(1, 0, 2).reshape(128, -1))
    sin_ar = np.ascontiguousarray(
        sin.reshape(NB, 128, HD // 2).transpose(1, 0, 2).reshape(128, -1))
    perm = _rope_perm()
    woT = np.ascontiguousarray(w_o.T)                      # [KVD, D]
    in_maps = []
    jrows = np.arange(D) // 128 % 2
    for r in range(8):
        b, kh = r // 4, r % 4
        heads = [g * KH + kh for g in range(4)]
        wq_sel = w_q.reshape(H, HD, D)[heads][:, perm, :]  # [4,128,D]
        wqT = np.ascontiguousarray(wq_sel.reshape(4 * HD, D).T)   # [D, 512]
        wkT = np.ascontiguousarray(w_k[kh * HD:(kh + 1) * HD][perm].T)  # [D,128]
        wvT = np.ascontiguousarray(w_v[kh * HD:(kh + 1) * HD].T)        # [D,128]
        sel = jrows == b
        in_maps.append({
            "x": x[b],
            "xstat": np.ascontiguousarray(x[b][kh * SQ:(kh + 1) * SQ]),
            "wq": np.ascontiguousarray(wqT[sel]),
            "wk": np.ascontiguousarray(wkT[sel]),
            "wv": np.ascontiguousarray(wvT[sel]),
            "wo": np.ascontiguousarray(woT[:, r * 256:(r + 1) * 256]),
            "cos": cos_ar, "sin": sin_ar,
            "qsel": qsel_host(b),
        })
    return in_maps


def _gains_trivial(inputs):
    return all(np.all(np.asarray(inputs[g]) == 1.0)
               for g in ("g_q", "g_k", "g_v", "g_o"))


def _numpy_fallback(inputs):
    """Faithful numpy reimplementation (slow); used only for unexpected configs."""
    x = np.asarray(inputs["x"], np.float32)
    cos, sin = (np.asarray(inputs[k], np.float32) for k in ("freq_cos", "freq_sin"))
    causal = int(np.asarray(inputs["causal"]))

    def rms(t, g):
        n = t * (1.0 / np.sqrt(np.mean(t * t, -1, keepdims=True, dtype=np.float32) + EPS))
        return (g * n).astype(np.float32)

    def actq(t):
        scale = 127.0 / np.clip(np.max(np.abs(t), -1, keepdims=True), 1e-4, None)
        q = np.round(t * scale)
        return np.clip(q, -128, 127) / scale

    def ternq(w):
        s = np.mean(np.abs(w), dtype=np.float32)
        return np.round(np.tanh(w / (s + EPS))) * np.arctanh(s)

    def lin(t, w, g):
        return actq(rms(t, g)).astype(np.float32) @ ternq(np.asarray(w, np.float32)).T

    Bb, Ss, Dd = x.shape
    q = lin(x, inputs["w_q"], np.asarray(inputs["g_q"], np.float32)).reshape(Bb, Ss, H, HD)
    k = lin(x, inputs["w_k"], np.asarray(inputs["g_k"], np.float32)).reshape(Bb, Ss, KH, HD)
    v = lin(x, inputs["w_v"], np.asarray(inputs["g_v"], np.float32)).reshape(Bb, Ss, KH, HD)

    def rope(t):
        t2 = t.reshape(*t.shape[:-1], -1, 2)
        c = cos[None, :, None, :]
        s_ = sin[None, :, None, :]
        o0 = t2[..., 0] * c - t2[..., 1] * s_
        o1 = t2[..., 0] * s_ + t2[..., 1] * c
        return np.stack([o0, o1], -1).reshape(t.shape).astype(np.float32)

    q, k = rope(q), rope(k)
    scale = np.float32(HD ** 0.5)
    q = q.transpose(0, 2, 1, 3) / scale
    k = k.transpose(0, 2, 1, 3)
    v = v.transpose(0, 2, 1, 3)
    qg = q.reshape(Bb, 4, KH, Ss, HD).sum(1)
    sc = np.einsum("bhnd,bhsd->bhns", qg, k).astype(np.float32)
    if causal:
        mask = np.tril(np.ones((Ss, Ss), bool))
        sc = np.where(mask[None, None], sc, np.float32(np.finfo(np.float32).min))
    sc = sc / scale
    sc = sc - sc.max(-1, keepdims=True)
    p = np.exp(sc)
    p /= p.sum(-1, keepdims=True)
    out = np.einsum("bhns,bhsd->bnhd", p, v).reshape(Bb, Ss, KVD)
    return lin(out, inputs["w_o"], np.asarray(inputs["g_o"], np.float32))


def kernel(**inputs):
    x = np.asarray(inputs["x"])
    if x.shape != (B, S, D) or not _gains_trivial(inputs):
        return _numpy_fallback(inputs)
    causal = bool(int(np.asarray(inputs["causal"])))
    key = ("bitattn", causal)
    if key not in _cache:
        _cache[key] = build(causal)
    nc = _cache[key]
    in_maps = _prep_inputs(inputs)
    res = run_bass_kernel_spmd(nc, in_maps, core_ids=list(range(8)))
    y = np.empty((B, S, D), np.float32)
    for r in range(8):
        b, qq = r // 4, r % 4
        for tb in range(4):
            blk = 4 * tb + qq
            y[b, blk * 128:(blk + 1) * 128, :] = res.results[r]["y"][tb * 128:(tb + 1) * 128]
    return y


if __name__ == "__main__":
    data = np.load("/tmp/inputs.npz")
    inputs = {k: data[k] for k in data.files}
    out = kernel(**inputs)
    exp = np.load("/tmp/expected.npy")
    err = np.linalg.norm(out - exp) / np.linalg.norm(exp)
    print("Relative error:", err)
